# revision 1
# baseline (speedup 1.0000x reference)
"""Trainium2 Bass kernel v4 for nn_AttentionFusion — fused stream, PE upsample.

Per batch element (B=8 -> one NeuronCore each):
    Q = ds(feat_wide), K = ds(feat_narrow)      # 2x2 center sums, [C, 4096]
    attn = softmax(Q^T K / (16 sqrt(C)))
    out = feat_wide + up4((attn @ K^T)^T)       # bilinear 4x upsample + residual

v4 layout: feat_wide streams through SBUF ONCE in 16-row blocks (80 MiB total
HBM traffic).  The whole upsample runs on the PE: for each normalized
attention sub-tile ot[tt] = [n=(2h,64w), c] the output rows are produced by
matmuls against 8 host-built [128, 512] matrices that fuse the 2-tap H-up
weights with the W-upsample matrix; bands of 2 output rows accumulate in PSUM
(cross-tile bands get one matmul from each neighbor tile), then a single DVE
add folds the streamed feat_wide rows in-place and the block is written out.

Engine mapping per n-block iteration (~22 us DMA slot):
  - PE: PV(nb-1) m-region-wise ahead of scores(nb) (single fp8 attn buffer,
    WAR-safe), band matmuls of tiles from nb-2 interleaved between regions
  - ScalarE: exp only (PSUM -> fp8 SBUF)
  - DVE: normalize (rcp + scale) + one [128,512] band add per 2 output rows
  - Pool: fw DMA waits + Q extraction (row-pair + col-pair sums)
  - Sync: fn chunk loads (prologue), fw block loads, out writes
"""

import math

import numpy as np


# ----------------------------------------------------------------------------
# numpy-side constants
# ----------------------------------------------------------------------------

def _build_upsample_matrix(n_in: int, n_out: int) -> np.ndarray:
    """U[h, H]: out[H] = sum_h U[h, H] * in[h] for torch-style bilinear,
    align_corners=False, antialias=False, scale n_out/n_in."""
    U = np.zeros((n_in, n_out), dtype=np.float64)
    scale = n_in / n_out
    for o in range(n_out):
        src = (o + 0.5) * scale - 0.5
        k0 = int(math.floor(src))
        frac = src - k0
        for k, wt in ((k0, 1.0 - frac), (k0 + 1, frac)):
            kc = min(max(k, 0), n_in - 1)
            U[kc, o] += wt
    return U


# Band weight matrices: band m covers output rows (2m, 2m+1).
#   m even (=2k): rows 4k+0,4k+1 = (0.375, 0.125) * y[k-1] + (0.625, 0.875) * y[k]
#   m odd (=2k+1): rows 4k+2,4k+3 = (0.875, 0.625) * y[k] + (0.125, 0.375) * y[k+1]
# y[k] lives in ot tile k//2 at h' = k%2.  Matrix M[(h',w), (r2,W)] =
# coef[h'][r2] * Uw[w, W] with Uw = 0.25 * U(64->256) (0.25 undoes the
# unscaled 2x2-sum downsample of K).
_BAND_COEFS = {
    "IN0": {0: (0.375, 0.125), 1: (0.625, 0.875)},  # m even, k odd: in-tile
    "IN1": {0: (0.875, 0.625), 1: (0.125, 0.375)},  # m odd, k even: in-tile
    "CEA": {1: (0.375, 0.125)},  # m even, k even: y[k-1] from tile tt-1
    "CEB": {0: (0.625, 0.875)},  # m even, k even: y[k] from tile tt
    "COA": {1: (0.875, 0.625)},  # m odd, k odd: y[k] from tile tt-1
    "COB": {0: (0.125, 0.375)},  # m odd, k odd: y[k+1] from tile tt
    "ET": {0: (1.0, 1.0)},       # band 0: rows 0,1 clamp to y[0]
    "EB": {1: (1.0, 1.0)},       # band 127: rows 254,255 clamp to y[63]
}
_BAND_NAMES = ("IN0", "IN1", "CEA", "CEB", "COA", "COB", "ET", "EB")


def _build_band_matrices() -> np.ndarray:
    """[8, 128, 512] f64: fused H-up x W-up band weight matrices."""
    Uw = _build_upsample_matrix(64, 256) * 0.25  # [64, 256]
    out = np.zeros((8, 128, 512), dtype=np.float64)
    for idx, name in enumerate(_BAND_NAMES):
        for hp, (c0, c1) in _BAND_COEFS[name].items():
            for r2, cf in ((0, c0), (1, c1)):
                out[idx, hp * 64 : hp * 64 + 64, r2 * 256 : r2 * 256 + 256] = (
                    cf * Uw
                )
    return out


# ----------------------------------------------------------------------------
# Bass kernel builder
# ----------------------------------------------------------------------------

def build_kernel():
    import concourse.bacc as bacc
    import concourse.bass as bass
    import concourse.mybir as mybir
    from concourse import tile

    f32 = mybir.dt.float32
    bf16 = mybir.dt.bfloat16
    f8 = mybir.dt.float8e4
    AOp = mybir.AluOpType
    ActFn = mybir.ActivationFunctionType

    C = 128          # channels = partitions
    HW = 256         # full resolution
    hw = 64          # downsampled resolution
    N = hw * hw      # 4096 attention positions
    MI = 32          # m tiles of 128
    NBLK = 8         # n blocks of 512
    NFB = 16         # fw row blocks of 16 rows
    EXP_SCALE = 1.0 / (16.0 * math.sqrt(C))
    MIDX = {n: i for i, n in enumerate(_BAND_NAMES)}

    nc = bacc.Bacc("TRN2", target_bir_lowering=False, debug=False)

    fw = nc.dram_tensor("feat_wide", [C, HW, HW], f32, kind="ExternalInput")
    fn = nc.dram_tensor("feat_narrow", [C, HW, HW], f32, kind="ExternalInput")
    bm = nc.dram_tensor("bandmat", [8, 128, 512], bf16, kind="ExternalInput")
    ident = nc.dram_tensor("ident", [128, 128], bf16, kind="ExternalInput")
    out = nc.dram_tensor("out", [C, HW, HW], f32, kind="ExternalOutput")

    with tile.TileContext(nc) as tc:
        with (
            tc.tile_pool(name="const", bufs=1) as const_pool,
            tc.tile_pool(name="big", bufs=1) as big_pool,
            tc.tile_pool(name="fwp", bufs=8) as fw_pool,
            tc.tile_pool(name="kch", bufs=2) as kch_pool,
            tc.tile_pool(name="qb", bufs=2) as qb_pool,
            tc.tile_pool(name="rs", bufs=2) as rs_pool,
            tc.tile_pool(name="sm", bufs=6) as sm_pool,
            tc.tile_pool(name="ps_s", bufs=2, space=bass.MemorySpace.PSUM) as ps_s,
            tc.tile_pool(name="ps_o", bufs=2, space=bass.MemorySpace.PSUM) as ps_o,
            tc.tile_pool(name="ps_b", bufs=2, space=bass.MemorySpace.PSUM) as ps_b,
        ):
            # ---- constants ----
            bm_t = const_pool.tile([128, 8, 512], bf16)
            for j in range(8):
                nc.sync.dma_start(bm_t[:, j, :], bm.ap()[j, :, :])
            id_t = const_pool.tile([128, 128], bf16)
            nc.sync.dma_start(id_t[:], ident[:, :])

            # ---- persistent buffers ----
            k_bf = big_pool.tile([C, N], bf16)
            kt1 = big_pool.tile([128, MI, 129], f8)   # K^T tiles + ones col
            nc.vector.memset(kt1[:], 1.0)
            at = big_pool.tile([128, MI, 512], f8)    # attn^T, single buffer

            fn3 = fn.ap().rearrange("c (i r) w -> c i (r w)", r=4)

            fw_tiles = {}
            q_tiles = {}
            ot_tiles = {}

            def load_fw(b):
                """Stream in fw rows 16b..16b+16 (sync queue dispatch)."""
                t = fw_pool.tile([C, 16, HW], f32, tag="fw")
                nc.sync.dma_start(t[:], fw.ap()[:, 16 * b : 16 * b + 16, :])
                fw_tiles[b] = t

            def q_extract(b):
                """Q columns of fw block b: rows 4i+1,4i+2, cols 4j+1,4j+2.
                Entirely on Pool so the DMA wait never blocks DVE."""
                t = fw_tiles[b]
                nbq, half = divmod(b, 2)
                if half == 0:
                    q_tiles[nbq] = qb_pool.tile([C, 512], bf16, tag="q", name="qt")
                qt = q_tiles[nbq]
                fw4 = t[:].rearrange("c (j r) w -> c j r w", r=4)
                rsum = rs_pool.tile([C, 4, 256], bf16, tag="qrs")
                nc.gpsimd.tensor_tensor(
                    rsum[:], fw4[:, :, 1, :], fw4[:, :, 2, :], AOp.add
                )
                rs4 = rsum[:].rearrange("c j (k f) -> c j k f", f=4)
                qv = qt[:, half * 256 : half * 256 + 256].rearrange(
                    "c (j k) -> c j k", k=hw
                )
                nc.gpsimd.tensor_tensor(qv, rs4[:, :, :, 1], rs4[:, :, :, 2], AOp.add)

            def load_k_chunk(cc):
                """fn rows 4i+1,4i+2 for i=4cc..4cc+4 -> k_bf cols + kt1 tiles."""
                ch = kch_pool.tile([C, 4, 512], f32, tag="kch")
                nc.sync.dma_start(ch[:], fn3[:, 4 * cc : 4 * cc + 4, 256:768])
                ch2 = ch[:].rearrange("c i (r w) -> c i r w", r=2)
                rsum = rs_pool.tile([C, 4, 256], bf16, tag="krs")
                nc.vector.tensor_tensor(rsum[:], ch2[:, :, 0, :], ch2[:, :, 1, :], AOp.add)
                rs4 = rsum[:].rearrange("c j (k f) -> c j k f", f=4)
                kv = k_bf[:, 256 * cc : 256 * cc + 256].rearrange(
                    "c (j k) -> c j k", k=hw
                )
                nc.vector.tensor_tensor(kv, rs4[:, :, :, 1], rs4[:, :, :, 2], AOp.add)
                for mi in (2 * cc, 2 * cc + 1):
                    pt = ps_b.tile([128, 512], f32, tag="bd")
                    ptb = pt[:, 0:64].bitcast(bf16)  # [128, 128] bf16 view
                    nc.tensor.transpose(
                        ptb, k_bf[:, 128 * mi : 128 * mi + 128], id_t[:]
                    )
                    nc.vector.tensor_copy(kt1[:, mi, 0:128], ptb)

            # ---- prologue: fw first (q gates the first scores), then K ----
            load_fw(0)
            load_fw(1)
            for cc in range(16):
                load_k_chunk(cc)
            q_extract(0)
            q_extract(1)

            def band_mm_list(m):
                """(ot tile, matrix name) pairs accumulating band m."""
                if m == 0:
                    return ((0, "ET"),)
                if m == 127:
                    return ((31, "EB"),)
                if m % 2 == 0:
                    k = m // 2
                    if k % 2 == 1:
                        return (((k - 1) // 2, "IN0"),)
                    return ((k // 2 - 1, "CEA"), (k // 2, "CEB"))
                k = (m - 1) // 2
                if k % 2 == 0:
                    return ((k // 2, "IN1"),)
                return (((k - 1) // 2, "COA"), ((k + 1) // 2, "COB"))

            def emit_band_mms(m, bd):
                mms = band_mm_list(m)
                for j, (tt, name) in enumerate(mms):
                    nc.tensor.matmul(
                        bd,
                        ot_tiles[tt][:],
                        bm_t[:, MIDX[name], :],
                        start=(j == 0),
                        stop=(j == len(mms) - 1),
                    )

            def emit_band(m):
                """Band m = output rows (2m, 2m+1): PE matmul(s) into PSUM,
                DVE add of streamed fw rows in place, write block when done."""
                bd = ps_b.tile([128, 512], f32, tag="bd")
                emit_band_mms(m, bd[:])
                b = m // 8
                lr = 2 * (m % 8)
                fwb = fw_tiles[b]
                nc.vector.tensor_tensor(
                    fwb[:, lr : lr + 2, :], bd[:], fwb[:, lr : lr + 2, :], AOp.add
                )
                if m % 8 == 7:
                    nc.sync.dma_start(
                        out.ap()[:, 16 * b : 16 * b + 16, :], fw_tiles.pop(b)[:]
                    )

            def bands_of_tail(tt):
                """Bands fully determined once ot[tt] exists."""
                lst = []
                if tt >= 1:
                    lst.append(4 * tt - 1)
                lst += [4 * tt, 4 * tt + 1, 4 * tt + 2]
                if tt == 31:
                    lst.append(127)
                return lst

            def norm_tail(nb_pv, ns, po):
                """normalize sub-tile ns of n-block nb_pv -> ot ring tile."""
                tt = 4 * nb_pv + ns
                rcp = sm_pool.tile([128, 1], f32, tag="rcp")
                nc.vector.reciprocal(rcp[:], po[:, 128:129])
                ot = sm_pool.tile([128, 128], bf16, tag="ot", name="ot_t")
                nc.vector.tensor_scalar(ot[:], po[:, 0:128], rcp[:], None, AOp.mult)
                ot_tiles[tt] = ot

            # ---- main loop ----
            # it(nb): PV(nb-1) + scores(nb) region-wise; bands of tiles
            # 4(nb-2)..4(nb-2)+3 interleaved between PV chunks; norms of
            # nb-1 at the end; q extraction for the next-next block last.
            for nb in range(NBLK):
                pos = None
                for b in (2 * nb + 2, 2 * nb + 3):
                    if b < NFB:
                        load_fw(b)
                if nb >= 1:
                    po0 = ps_o.tile([128, 2, 129], f32, tag="po")
                    po1 = ps_o.tile([128, 2, 129], f32, tag="po")
                    pos = (po0[:, 0, :], po0[:, 1, :], po1[:, 0, :], po1[:, 1, :])
                def pv_part(mi0, mi1):
                    for ns in range(4):
                        for mi in range(mi0, mi1):
                            nc.tensor.matmul(
                                pos[ns],
                                at[:, mi, 128 * ns : 128 * ns + 128],
                                kt1[:, mi, :],
                                start=(mi == 0),
                                stop=(mi == MI - 1),
                            )

                def band_chunk(i):
                    for m in bands_of_tail(4 * (nb - 2) + i):
                        emit_band(m)

                qt = q_tiles[nb]

                def spair(mp):
                    ps = ps_s.tile([128, 2, 512], f32, tag="ps")
                    for s in range(2):
                        mi = 2 * mp + s
                        nc.tensor.matmul(
                            ps[:, s, :],
                            k_bf[:, 128 * mi : 128 * mi + 128],
                            qt[:],
                            start=True,
                            stop=True,
                        )
                    nc.scalar.activation(
                        at[:, 2 * mp : 2 * mp + 2, :],
                        ps[:],
                        ActFn.Exp,
                        bias=0.0,
                        scale=EXP_SCALE,
                    )

                # fine-grained weave: PV(nb-1) is packed into the first 8
                # m-pair slots (4 m-tiles each, still rotating over the 4
                # PSUM accumulators), norms fire at mp 7, and the bands of
                # the just-normalized tails fill slots 8..15 — so output
                # blocks are written only half an iteration after their last
                # attention column, and exps (the ScalarE pacer) stream
                # continuously from ~1 us into the iteration.
                bandq = []
                if nb >= 2:
                    bandq += list(bands_of_tail(4 * (nb - 2) + 2))
                    bandq += list(bands_of_tail(4 * (nb - 2) + 3))
                backq = []
                if pos is not None:
                    backq += list(bands_of_tail(4 * (nb - 1)))
                    backq += list(bands_of_tail(4 * (nb - 1) + 1))
                for mp in range(16):
                    if pos is not None and mp < 8:
                        pv_part(4 * mp, 4 * mp + 4)
                        if mp == 7:
                            for ns in range(4):
                                norm_tail(nb - 1, ns, pos[ns])
                            bandq += backq
                    spair(mp)
                    if bandq:
                        emit_band(bandq.pop(0))
                while bandq:
                    emit_band(bandq.pop(0))
                for b in (2 * nb + 2, 2 * nb + 3):
                    if b < NFB:
                        q_extract(b)

            # ---- epilogue: PV(7) + leftover tails 26,27 interleaved ----
            po0 = ps_o.tile([128, 2, 129], f32, tag="po")
            po1 = ps_o.tile([128, 2, 129], f32, tag="po")
            epos = (po0[:, 0, :], po0[:, 1, :], po1[:, 0, :], po1[:, 1, :])
            ebandq = list(bands_of_tail(26)) + list(bands_of_tail(27))
            for i in range(4):
                for ns in range(4):
                    for mi in range(8 * i, 8 * i + 8):
                        nc.tensor.matmul(
                            epos[ns],
                            at[:, mi, 128 * ns : 128 * ns + 128],
                            kt1[:, mi, :],
                            start=(mi == 0),
                            stop=(mi == MI - 1),
                        )
                    if i == 3:
                        norm_tail(NBLK - 1, ns, epos[ns])
                    if i % 2 == 1 and ebandq:
                        emit_band(ebandq.pop(0))
            while ebandq:
                emit_band(ebandq.pop(0))
            # tail bands 111..127: 111 completes block 13; pair the rest into
            # 2-band ps_s tiles (scores are done, those banks are free) so
            # one DVE add covers 4 output rows — halves PE<->DVE round trips
            emit_band(111)
            for m0 in range(112, 128, 2):
                bd2 = ps_s.tile([128, 2, 512], f32, tag="ps")
                emit_band_mms(m0, bd2[:, 0, :])
                emit_band_mms(m0 + 1, bd2[:, 1, :])
                b = m0 // 8
                lr = 2 * (m0 % 8)
                fwb = fw_tiles[b]
                nc.vector.tensor_tensor(
                    fwb[:, lr : lr + 4, :], bd2[:], fwb[:, lr : lr + 4, :], AOp.add
                )
                if (m0 + 1) % 8 == 7:
                    nc.sync.dma_start(
                        out.ap()[:, 16 * b : 16 * b + 16, :], fw_tiles.pop(b)[:]
                    )

    nc.compile()
    return nc


_NC_CACHE = None


def _get_nc():
    global _NC_CACHE
    if _NC_CACHE is None:
        _NC_CACHE = build_kernel()
    return _NC_CACHE


def run(feat_wide: np.ndarray, feat_narrow: np.ndarray, trace: bool = False):
    """Run on 8 NeuronCores; returns (output [8,128,256,256], BassKernelResults)."""
    from concourse.bass_utils import run_bass_kernel_spmd
    import ml_dtypes

    B, C, H, W = feat_wide.shape
    assert (B, C, H, W) == (8, 128, 256, 256)

    bandmat = _build_band_matrices().astype(ml_dtypes.bfloat16)
    identity = np.eye(128, dtype=ml_dtypes.bfloat16)

    nc = _get_nc()
    in_maps = [
        {
            "feat_wide": np.ascontiguousarray(np.asarray(feat_wide[b], dtype=np.float32)),
            "feat_narrow": np.ascontiguousarray(np.asarray(feat_narrow[b], dtype=np.float32)),
            "bandmat": bandmat,
            "ident": identity,
        }
        for b in range(B)
    ]
    res = run_bass_kernel_spmd(nc, in_maps, core_ids=list(range(8)), trace=trace)
    out = np.stack([res.results[b]["out"] for b in range(B)], axis=0)
    return out, res


def kernel(feat_wide: np.ndarray, feat_narrow: np.ndarray) -> np.ndarray:
    out, _ = run(feat_wide, feat_narrow, trace=False)
    return out



# revision 7
# speedup vs baseline: 1.4120x; 1.4120x over previous
"""Trainium2 Bass kernel v5 for nn_AttentionFusion — fp16 staging, fused bands,
in-iteration PV.

Per batch element (B=8 -> one NeuronCore each):
    Q = ds(feat_wide), K = ds(feat_narrow)      # 2x2 center sums, [C, 4096]
    attn = softmax(Q^T K / (16 sqrt(C)))
    out = feat_wide + up4((attn @ K^T)^T)       # bilinear 4x upsample + residual

v5 changes vs v4:
  - fw / fn / out staged in DRAM as fp16 (host casts): HBM traffic 85 -> 42 MB.
  - Cross-tile bands fused: partition-shifted Z tiles (built by SBUF->SBUF
    DMA after the norms) let every band be a single matmul (128 total).
  - PV(nb) runs IN iteration nb, trailing exp by 2 m-pair slots; norms at
    iteration end; bands of the just-normalized tiles run next iteration.
    Pipeline depth ~1 iteration; short epilogue.
  - PE warmup matmuls at t~0 so the HAM clock gate is at 8/8 when scores
    start; kt1 PSUM->SBUF casts on ScalarE (idle during prologue).

Engine mapping steady state (per n-block iteration ~21 us):
  - PE: 32 score mms + 128 PV mms + 16 band mms (+ kt1 transposes, iter 0)
  - ScalarE: exp per m-pair (PSUM -> fp8 at)
  - DVE: band adds (PSUM + fw fp16 -> fp16), norms, k-chunk sums (prologue)
  - GpSimd: Q extraction (fp16 strided sums)
  - Sync: all DMA dispatch (loads, Z copies, out writes)
"""

import math

import numpy as np


# ----------------------------------------------------------------------------
# numpy-side constants
# ----------------------------------------------------------------------------

def _build_upsample_matrix(n_in: int, n_out: int) -> np.ndarray:
    """U[h, H]: out[H] = sum_h U[h, H] * in[h] for torch-style bilinear,
    align_corners=False, antialias=False, scale n_out/n_in."""
    U = np.zeros((n_in, n_out), dtype=np.float64)
    scale = n_in / n_out
    for o in range(n_out):
        src = (o + 0.5) * scale - 0.5
        k0 = int(math.floor(src))
        frac = src - k0
        for k, wt in ((k0, 1.0 - frac), (k0 + 1, frac)):
            kc = min(max(k, 0), n_in - 1)
            U[kc, o] += wt
    return U


# Band m covers output rows (2m, 2m+1).  With y[k] the H-downsampled rows,
# H-up row 4k+r draws from y via 2 taps.  ot tile t holds y rows (2t, 2t+1)
# as partitions (h'=0/1) x 64 w; Z tile t holds [ot_t h'=1 ; ot_{t+1} h'=0]
# so every band is ONE matmul:
#   m=4t+1: IN1 on ot_t     rows (4k+2,4k+3), k=2t   even
#   m=4t+2: IN0 on ot_t     rows (4k,  4k+1), k=2t+1 odd
#   m=4t+3: ZO  on Z_t      rows (4k+2,4k+3), k=2t+1 odd
#   m=4t+4: ZE  on Z_t      rows (4k,  4k+1), k=2t+2 even
#   m=0: ET on ot_0; m=127: EB on ot_31 (edge clamps)
# Matrix[(h'',w),(r2,W)] = coef[h''][r2] * Uw[w,W], Uw = 0.25*U(64->256)
# (0.25 undoes the unscaled 2x2-sum downsample of K).
_BAND_COEFS = {
    "IN0": {0: (0.375, 0.125), 1: (0.625, 0.875)},  # h'0=y[k-1], h'1=y[k]
    "IN1": {0: (0.875, 0.625), 1: (0.125, 0.375)},  # h'0=y[k],   h'1=y[k+1]
    "ZE": {0: (0.375, 0.125), 1: (0.625, 0.875)},   # low=y[k-1], up=y[k]
    "ZO": {0: (0.875, 0.625), 1: (0.125, 0.375)},   # low=y[k],   up=y[k+1]
    "ET": {0: (1.0, 1.0)},                          # rows 0,1 clamp to y[0]
    "EB": {1: (1.0, 1.0)},                          # rows 254,255 -> y[63]
}
_BAND_NAMES = ("IN0", "IN1", "ZE", "ZO", "ET", "EB")


def _build_band_matrices() -> np.ndarray:
    """[6, 128, 512] f64: fused H-up x W-up band weight matrices."""
    Uw = _build_upsample_matrix(64, 256) * 0.25  # [64, 256]
    out = np.zeros((6, 128, 512), dtype=np.float64)
    for idx, name in enumerate(_BAND_NAMES):
        for hp, (c0, c1) in _BAND_COEFS[name].items():
            for r2, cf in ((0, c0), (1, c1)):
                out[idx, hp * 64 : hp * 64 + 64, r2 * 256 : r2 * 256 + 256] = (
                    cf * Uw
                )
    return out


# ----------------------------------------------------------------------------
# Bass kernel builder
# ----------------------------------------------------------------------------

def build_kernel():
    import concourse.bacc as bacc
    import concourse.bass as bass
    import concourse.mybir as mybir
    from concourse import tile

    f32 = mybir.dt.float32
    f16 = mybir.dt.float16
    bf16 = mybir.dt.bfloat16
    f8 = mybir.dt.float8e4
    AOp = mybir.AluOpType
    ActFn = mybir.ActivationFunctionType

    C = 128          # channels = partitions
    HW = 256         # full resolution
    hw = 64          # downsampled resolution
    N = hw * hw      # 4096 attention positions
    MI = 32          # m tiles of 128
    NBLK = 8         # n blocks of 512
    NFB = 16         # fw row blocks of 16 rows
    EXP_SCALE = 1.0 / (16.0 * math.sqrt(C))
    MIDX = {n: i for i, n in enumerate(_BAND_NAMES)}

    nc = bacc.Bacc("TRN2", target_bir_lowering=False, debug=False)

    fw = nc.dram_tensor("feat_wide", [C, HW, HW], f16, kind="ExternalInput")
    fn = nc.dram_tensor("feat_narrow", [C, HW, HW], f16, kind="ExternalInput")
    bm = nc.dram_tensor("bandmat", [6, 128, 512], bf16, kind="ExternalInput")
    ident = nc.dram_tensor("ident", [128, 128], bf16, kind="ExternalInput")
    out = nc.dram_tensor("out", [C, HW, HW], f16, kind="ExternalOutput")

    with tile.TileContext(nc) as tc:
        with (
            tc.tile_pool(name="const", bufs=1) as const_pool,
            tc.tile_pool(name="big", bufs=1) as big_pool,
            tc.tile_pool(name="fwp", bufs=8) as fw_pool,
            tc.tile_pool(name="kch", bufs=3) as kch_pool,
            tc.tile_pool(name="qb", bufs=2) as qb_pool,
            tc.tile_pool(name="rs", bufs=2) as rs_pool,
            tc.tile_pool(name="ot", bufs=2) as ot_pool,
            tc.tile_pool(name="zt", bufs=2) as zt_pool,
            tc.tile_pool(name="sm", bufs=4) as sm_pool,
            tc.tile_pool(name="ps_s", bufs=2, space=bass.MemorySpace.PSUM) as ps_s,
            tc.tile_pool(name="ps_o", bufs=2, space=bass.MemorySpace.PSUM) as ps_o,
            tc.tile_pool(name="ps_b", bufs=2, space=bass.MemorySpace.PSUM) as ps_b,
        ):
            # ---- constants ----
            bm_t = const_pool.tile([128, 6, 512], bf16)
            for j in range(6):
                nc.sync.dma_start(bm_t[:, j, :], bm.ap()[j, :, :])
            id_t = const_pool.tile([128, 128], bf16)
            nc.sync.dma_start(id_t[:], ident[:, :])

            # ---- persistent buffers ----
            k_bf = big_pool.tile([C, N], bf16)
            kt1 = big_pool.tile([128, MI, 129], f8)   # K^T tiles + ones col
            nc.vector.memset(kt1[:], 1.0)
            at = big_pool.tile([128, MI, 512], f8)    # attn^T (exp'd scores)

            fn3 = fn.ap().rearrange("c (i r) w -> c i (r w)", r=4)

            fw_tiles = {}
            q_tiles = {}
            ot_tiles = {}   # nb -> [128, 4, 128] tile (y sub-tiles of block nb)
            zt_tiles = {}   # nb -> [128, 4, 128] tile (slot s = Z_{4nb-1+s})

            # ---- PE warmup: keep the HAM clock-gate busy from t~0 so the
            # first real matmuls run at 2.4 GHz.  Junk mms on the identity.
            pwarm = ps_b.tile([128, 512], f32, tag="bd")
            for _ in range(20):
                nc.tensor.matmul(
                    pwarm[:, 0:128], id_t[:], id_t[:], start=True, stop=True
                )

            def load_fw(b):
                """Stream in fw rows 16b..16b+16 (fp16)."""
                t = fw_pool.tile([C, 16, HW], f16, tag="fw")
                nc.sync.dma_start(t[:], fw.ap()[:, 16 * b : 16 * b + 16, :])
                fw_tiles[b] = t

            def q_extract(b):
                """Q columns of fw block b: rows 4i+1,4i+2, cols 4j+1,4j+2.
                On GpSimd (no PSUM need, keeps DVE free)."""
                t = fw_tiles[b]
                nbq, half = divmod(b, 2)
                if half == 0:
                    q_tiles[nbq] = qb_pool.tile([C, 512], bf16, tag="q", name="qt")
                qt = q_tiles[nbq]
                fw4 = t[:].rearrange("c (j r) w -> c j r w", r=4)
                rsum = rs_pool.tile([C, 4, 256], bf16, tag="qrs")
                nc.gpsimd.tensor_tensor(
                    rsum[:], fw4[:, :, 1, :], fw4[:, :, 2, :], AOp.add
                )
                rs4 = rsum[:].rearrange("c j (k f) -> c j k f", f=4)
                qv = qt[:, half * 256 : half * 256 + 256].rearrange(
                    "c (j k) -> c j k", k=hw
                )
                nc.gpsimd.tensor_tensor(qv, rs4[:, :, :, 1], rs4[:, :, :, 2], AOp.add)

            def load_k_chunk(cc):
                """fn rows 4i+1,4i+2 for i=4cc..4cc+4 -> k_bf cols (DVE sums).
                kt1 transposes/casts are emitted later, inside iteration 0,
                so they don't head-block the PE/ScalarE FIFOs."""
                ch = kch_pool.tile([C, 4, 512], f16, tag="kch")
                nc.sync.dma_start(ch[:], fn3[:, 4 * cc : 4 * cc + 4, 256:768])
                ch2 = ch[:].rearrange("c i (r w) -> c i r w", r=2)
                rsum = rs_pool.tile([C, 4, 256], bf16, tag="krs")
                nc.vector.tensor_tensor(rsum[:], ch2[:, :, 0, :], ch2[:, :, 1, :], AOp.add)
                rs4 = rsum[:].rearrange("c j (k f) -> c j k f", f=4)
                kv = k_bf[:, 256 * cc : 256 * cc + 256].rearrange(
                    "c (j k) -> c j k", k=hw
                )
                nc.vector.tensor_tensor(kv, rs4[:, :, :, 1], rs4[:, :, :, 2], AOp.add)

            def make_kt1(cc):
                """Transpose chunk cc's k_bf tiles into kt1 (PE + ScalarE)."""
                for mi in (2 * cc, 2 * cc + 1):
                    pt = ps_b.tile([128, 512], f32, tag="bd")
                    ptb = pt[:, 0:64].bitcast(bf16)  # [128, 128] bf16 view
                    nc.tensor.transpose(
                        ptb, k_bf[:, 128 * mi : 128 * mi + 128], id_t[:]
                    )
                    nc.scalar.copy(kt1[:, mi, 0:128], ptb)

            # ---- band machinery ----
            def band_source(m):
                """(kind, t, matrix name) for band m; kind 'ot' or 'z'."""
                if m == 0:
                    return ("ot", 0, "ET")
                if m == 127:
                    return ("ot", 31, "EB")
                if m % 2 == 0:
                    k = m // 2
                    if k % 2 == 1:
                        return ("ot", (k - 1) // 2, "IN0")
                    return ("z", k // 2 - 1, "ZE")
                k = (m - 1) // 2
                if k % 2 == 0:
                    return ("ot", k // 2, "IN1")
                return ("z", (k - 1) // 2, "ZO")

            def band_stationary(m):
                kind, t, name = band_source(m)
                if kind == "ot":
                    return ot_tiles[t // 4][:, t % 4, :], name
                return zt_tiles[(t + 1) // 4][:, (t + 1) % 4, :], name

            def emit_band(m):
                """Band m = output rows (2m, 2m+1): one PE matmul into PSUM,
                DVE add of fw rows in place (fp16), write block when done."""
                st, name = band_stationary(m)
                bd = ps_b.tile([128, 512], f32, tag="bd")
                nc.tensor.matmul(
                    bd[:], st, bm_t[:, MIDX[name], :], start=True, stop=True
                )
                b = m // 8
                lr = 2 * (m % 8)
                fwb = fw_tiles[b]
                nc.vector.tensor_tensor(
                    fwb[:, lr : lr + 2, :], bd[:], fwb[:, lr : lr + 2, :], AOp.add
                )
                if m % 8 == 7:
                    nc.sync.dma_start(
                        out.ap()[:, 16 * b : 16 * b + 16, :], fw_tiles.pop(b)[:]
                    )

            def make_norms(nb, pos):
                """Normalize the 4 sub-tiles of block nb -> ot tile."""
                otb = ot_pool.tile([128, 4, 128], bf16, tag="ot", name="ot_t")
                ot_tiles[nb] = otb
                for ns in range(4):
                    po = pos[ns]
                    rcp = sm_pool.tile([128, 1], f32, tag="rcp")
                    nc.vector.reciprocal(rcp[:], po[:, 128:129])
                    nc.vector.tensor_scalar(
                        otb[:, ns, :], po[:, 0:128], rcp[:], None, AOp.mult
                    )

            def make_z(nb):
                """Z tiles for block nb: slot s = Z_{4nb-1+s} =
                [y[2t+1] ; y[2t+2]] with t = 4nb-1+s.  SBUF->SBUF DMAs."""
                zb = zt_pool.tile([128, 4, 128], bf16, tag="zt", name="zt_t")
                zt_tiles[nb] = zb
                otb = ot_tiles[nb]
                # upper halves: Z_t[64:128] = ot_{t+1}[0:64] for all 4 slots
                nc.sync.dma_start(zb[64:128, 0:4, :], otb[0:64, 0:4, :])
                # lower halves: Z_t[0:64] = ot_t[64:128]
                nc.sync.dma_start(zb[0:64, 1:4, :], otb[64:128, 0:3, :])
                if nb > 0:
                    nc.sync.dma_start(
                        zb[0:64, 0, :], ot_tiles[nb - 1][64:128, 3, :]
                    )

            # ---- prologue ----
            # Dispatch order on sync: fw0, fw1 first (gate qt(0)), then fn
            # chunks (gate the scores pacing), fw2, fw3 interleaved.
            load_fw(0)
            load_fw(1)
            for cc in range(4):
                load_k_chunk(cc)
            load_fw(2)
            load_fw(3)
            for cc in range(4, 16):
                load_k_chunk(cc)
            q_extract(0)
            q_extract(1)

            bandq = []

            def band_step():
                if bandq:
                    emit_band(bandq.pop(0))

            # ---- main loop ----
            for nb in range(NBLK):
                po0 = ps_o.tile([128, 2, 129], f32, tag="po")
                po1 = ps_o.tile([128, 2, 129], f32, tag="po")
                pos = (po0[:, 0, :], po0[:, 1, :], po1[:, 0, :], po1[:, 1, :])
                qt = q_tiles[nb]

                def spair(mp):
                    ps = ps_s.tile([128, 2, 512], f32, tag="ps")
                    for s in range(2):
                        mi = 2 * mp + s
                        nc.tensor.matmul(
                            ps[:, s, :],
                            k_bf[:, 128 * mi : 128 * mi + 128],
                            qt[:],
                            start=True,
                            stop=True,
                        )
                    nc.scalar.activation(
                        at[:, 2 * mp : 2 * mp + 2, :],
                        ps[:],
                        ActFn.Exp,
                        bias=0.0,
                        scale=EXP_SCALE,
                    )

                def pv_chunk(mp):
                    for ns in range(4):
                        for mi in (2 * mp, 2 * mp + 1):
                            nc.tensor.matmul(
                                pos[ns],
                                at[:, mi, 128 * ns : 128 * ns + 128],
                                kt1[:, mi, :],
                                start=(mi == 0),
                                stop=(mi == MI - 1),
                            )

                for mp in range(16):
                    if nb == 0:
                        make_kt1(mp)
                    spair(mp)
                    if mp >= 2:
                        pv_chunk(mp - 2)
                    band_step()
                    if mp == 0:
                        if 2 * nb + 4 < NFB:
                            load_fw(2 * nb + 4)
                    if mp == 7:
                        if 2 * nb + 2 < NFB:
                            q_extract(2 * nb + 2)
                    if mp == 8:
                        if 2 * nb + 5 < NFB:
                            load_fw(2 * nb + 5)
                    if mp == 15:
                        if 2 * nb + 3 < NFB:
                            q_extract(2 * nb + 3)
                pv_chunk(14)
                band_step()
                pv_chunk(15)
                band_step()
                make_norms(nb, pos)
                make_z(nb)
                # newly enabled bands: contiguous run 16nb-1 .. 16nb+14
                lo = max(0, 16 * nb - 1)
                bandq.extend(range(lo, 16 * nb + 15))

            # ---- epilogue: remaining bands (tiles 28..31 & Z 27..30).
            # Scores are done, so ps_s banks are free: pair bands into
            # [128,2,512] tiles -> one DVE add per 4 output rows.
            emit_band(bandq.pop(0))  # band 111 completes block 13
            for m0 in range(112, 128, 2):
                bd2 = ps_s.tile([128, 2, 512], f32, tag="ps")
                for j, m in enumerate((m0, m0 + 1)):
                    st, name = band_stationary(m)
                    nc.tensor.matmul(
                        bd2[:, j, :], st, bm_t[:, MIDX[name], :],
                        start=True, stop=True,
                    )
                b = m0 // 8
                lr = 2 * (m0 % 8)
                fwb = fw_tiles[b]
                nc.vector.tensor_tensor(
                    fwb[:, lr : lr + 4, :], bd2[:], fwb[:, lr : lr + 4, :], AOp.add
                )
                if (m0 + 1) % 8 == 7:
                    nc.sync.dma_start(
                        out.ap()[:, 16 * b : 16 * b + 16, :], fw_tiles.pop(b)[:]
                    )

    nc.compile()
    return nc


_NC_CACHE = None


def _get_nc():
    global _NC_CACHE
    if _NC_CACHE is None:
        _NC_CACHE = build_kernel()
    return _NC_CACHE


def run(feat_wide: np.ndarray, feat_narrow: np.ndarray, trace: bool = False):
    """Run on 8 NeuronCores; returns (output [8,128,256,256], BassKernelResults)."""
    from concourse.bass_utils import run_bass_kernel_spmd
    import ml_dtypes

    B, C, H, W = feat_wide.shape
    assert (B, C, H, W) == (8, 128, 256, 256)

    bandmat = _build_band_matrices().astype(ml_dtypes.bfloat16)
    identity = np.eye(128, dtype=ml_dtypes.bfloat16)

    fw16 = np.asarray(feat_wide, dtype=np.float16)
    fn16 = np.asarray(feat_narrow, dtype=np.float16)

    nc = _get_nc()
    in_maps = [
        {
            "feat_wide": np.ascontiguousarray(fw16[b]),
            "feat_narrow": np.ascontiguousarray(fn16[b]),
            "bandmat": bandmat,
            "ident": identity,
        }
        for b in range(B)
    ]
    res = run_bass_kernel_spmd(nc, in_maps, core_ids=list(range(8)), trace=trace)
    outv = np.stack(
        [np.asarray(res.results[b]["out"], dtype=np.float32) for b in range(B)],
        axis=0,
    )
    return outv, res


def kernel(feat_wide: np.ndarray, feat_narrow: np.ndarray) -> np.ndarray:
    out, _ = run(feat_wide, feat_narrow, trace=False)
    return out


# revision 13
# speedup vs baseline: 1.4816x; 1.0492x over previous
"""Trainium2 Bass kernel v5 for nn_AttentionFusion — fp16 staging, fused bands,
in-iteration PV.

Per batch element (B=8 -> one NeuronCore each):
    Q = ds(feat_wide), K = ds(feat_narrow)      # 2x2 center sums, [C, 4096]
    attn = softmax(Q^T K / (16 sqrt(C)))
    out = feat_wide + up4((attn @ K^T)^T)       # bilinear 4x upsample + residual

v5 changes vs v4:
  - fw / fn / out staged in DRAM as fp16 (host casts): HBM traffic 85 -> 42 MB.
  - Cross-tile bands fused: partition-shifted Z tiles (built by SBUF->SBUF
    DMA after the norms) let every band be a single matmul (128 total).
  - PV(nb) runs IN iteration nb, trailing exp by 2 m-pair slots; norms at
    iteration end; bands of the just-normalized tiles run next iteration.
    Pipeline depth ~1 iteration; short epilogue.
  - PE warmup matmuls at t~0 so the HAM clock gate is at 8/8 when scores
    start; kt1 PSUM->SBUF casts on ScalarE (idle during prologue).

Engine mapping steady state (per n-block iteration ~21 us):
  - PE: 32 score mms + 128 PV mms + 16 band mms (+ kt1 transposes, iter 0)
  - ScalarE: exp per m-pair (PSUM -> fp8 at)
  - DVE: band adds (PSUM + fw fp16 -> fp16), norms, k-chunk sums (prologue)
  - GpSimd: Q extraction (fp16 strided sums)
  - Sync: all DMA dispatch (loads, Z copies, out writes)
"""

import math

import numpy as np


# ----------------------------------------------------------------------------
# numpy-side constants
# ----------------------------------------------------------------------------

def _build_upsample_matrix(n_in: int, n_out: int) -> np.ndarray:
    """U[h, H]: out[H] = sum_h U[h, H] * in[h] for torch-style bilinear,
    align_corners=False, antialias=False, scale n_out/n_in."""
    U = np.zeros((n_in, n_out), dtype=np.float64)
    scale = n_in / n_out
    for o in range(n_out):
        src = (o + 0.5) * scale - 0.5
        k0 = int(math.floor(src))
        frac = src - k0
        for k, wt in ((k0, 1.0 - frac), (k0 + 1, frac)):
            kc = min(max(k, 0), n_in - 1)
            U[kc, o] += wt
    return U


# Band m covers output rows (2m, 2m+1).  With y[k] the H-downsampled rows,
# H-up row 4k+r draws from y via 2 taps.  ot tile t holds y rows (2t, 2t+1)
# as partitions (h'=0/1) x 64 w; Z tile t holds [ot_t h'=1 ; ot_{t+1} h'=0]
# so every band is ONE matmul:
#   m=4t+1: IN1 on ot_t     rows (4k+2,4k+3), k=2t   even
#   m=4t+2: IN0 on ot_t     rows (4k,  4k+1), k=2t+1 odd
#   m=4t+3: ZO  on Z_t      rows (4k+2,4k+3), k=2t+1 odd
#   m=4t+4: ZE  on Z_t      rows (4k,  4k+1), k=2t+2 even
#   m=0: ET on ot_0; m=127: EB on ot_31 (edge clamps)
# Matrix[(h'',w),(r2,W)] = coef[h''][r2] * Uw[w,W], Uw = 0.25*U(64->256)
# (0.25 undoes the unscaled 2x2-sum downsample of K).
_BAND_COEFS = {
    "IN0": {0: (0.375, 0.125), 1: (0.625, 0.875)},  # h'0=y[k-1], h'1=y[k]
    "IN1": {0: (0.875, 0.625), 1: (0.125, 0.375)},  # h'0=y[k],   h'1=y[k+1]
    "ZE": {0: (0.375, 0.125), 1: (0.625, 0.875)},   # low=y[k-1], up=y[k]
    "ZO": {0: (0.875, 0.625), 1: (0.125, 0.375)},   # low=y[k],   up=y[k+1]
    "ET": {0: (1.0, 1.0)},                          # rows 0,1 clamp to y[0]
    "EB": {1: (1.0, 1.0)},                          # rows 254,255 -> y[63]
}
_BAND_NAMES = ("IN0", "IN1", "ZE", "ZO", "ET", "EB")


def _build_band_matrices() -> np.ndarray:
    """[6, 128, 512] f64: fused H-up x W-up band weight matrices."""
    Uw = _build_upsample_matrix(64, 256) * 0.25  # [64, 256]
    out = np.zeros((6, 128, 512), dtype=np.float64)
    for idx, name in enumerate(_BAND_NAMES):
        for hp, (c0, c1) in _BAND_COEFS[name].items():
            for r2, cf in ((0, c0), (1, c1)):
                out[idx, hp * 64 : hp * 64 + 64, r2 * 256 : r2 * 256 + 256] = (
                    cf * Uw
                )
    return out


# ----------------------------------------------------------------------------
# Bass kernel builder
# ----------------------------------------------------------------------------

def build_kernel():
    import concourse.bacc as bacc
    import concourse.bass as bass
    import concourse.mybir as mybir
    from concourse import tile

    f32 = mybir.dt.float32
    f16 = mybir.dt.float16
    bf16 = mybir.dt.bfloat16
    f8 = mybir.dt.float8e4
    AOp = mybir.AluOpType
    ActFn = mybir.ActivationFunctionType

    C = 128          # channels = partitions
    HW = 256         # full resolution
    hw = 64          # downsampled resolution
    N = hw * hw      # 4096 attention positions
    MI = 32          # m tiles of 128
    NBLK = 8         # n blocks of 512
    NFB = 16         # fw row blocks of 16 rows
    EXP_SCALE = 1.0 / (16.0 * math.sqrt(C))
    MIDX = {n: i for i, n in enumerate(_BAND_NAMES)}

    nc = bacc.Bacc("TRN2", target_bir_lowering=False, debug=False)

    fw = nc.dram_tensor("feat_wide", [C, HW, HW], f16, kind="ExternalInput")
    fn = nc.dram_tensor("feat_narrow", [C, HW, HW], f16, kind="ExternalInput")
    bm = nc.dram_tensor("bandmat", [6, 128, 512], bf16, kind="ExternalInput")
    ident = nc.dram_tensor("ident", [128, 128], bf16, kind="ExternalInput")
    out = nc.dram_tensor("out", [C, HW, HW], f16, kind="ExternalOutput")

    with tile.TileContext(nc) as tc:
        with (
            tc.tile_pool(name="const", bufs=1) as const_pool,
            tc.tile_pool(name="big", bufs=1) as big_pool,
            tc.tile_pool(name="fwp", bufs=8) as fw_pool,
            tc.tile_pool(name="kch", bufs=4) as kch_pool,
            tc.tile_pool(name="qb", bufs=2) as qb_pool,
            tc.tile_pool(name="rs", bufs=2) as rs_pool,
            tc.tile_pool(name="ot", bufs=2) as ot_pool,
            tc.tile_pool(name="zt", bufs=2) as zt_pool,
            tc.tile_pool(name="sm", bufs=4) as sm_pool,
            tc.tile_pool(name="ps_s", bufs=2, space=bass.MemorySpace.PSUM) as ps_s,
            tc.tile_pool(name="ps_o", bufs=2, space=bass.MemorySpace.PSUM) as ps_o,
            tc.tile_pool(name="ps_b", bufs=2, space=bass.MemorySpace.PSUM) as ps_b,
        ):
            # ---- constants (ident first: it gates the PE warmup) ----
            id_t = const_pool.tile([128, 128], bf16)
            nc.sync.dma_start(id_t[:], ident[:, :])
            bm_t = const_pool.tile([128, 6, 512], bf16)

            # ---- persistent buffers ----
            k_bf = big_pool.tile([C, N], bf16)
            kt1 = big_pool.tile([128, MI, 129], f8)   # K^T tiles + ones col
            nc.vector.memset(kt1[:], 1.0)
            at = big_pool.tile([128, MI, 512], f8)    # attn^T (exp'd scores)

            fn3 = fn.ap().rearrange("c (i r) w -> c i (r w)", r=4)

            fw_tiles = {}
            q_tiles = {}
            ot_tiles = {}   # nb -> [128, 4, 128] tile (y sub-tiles of block nb)
            zt_tiles = {}   # nb -> [128, 4, 128] tile (slot s = Z_{4nb-1+s})

            # ---- PE warmup: keep the HAM clock-gate busy from t~0 so the
            # first real matmuls run at 2.4 GHz.  Junk mms on the identity.
            pwarm = ps_b.tile([128, 512], f32, tag="bd")
            for _ in range(20):
                nc.tensor.matmul(
                    pwarm[:, 0:128], id_t[:], id_t[:], start=True, stop=True
                )

            def load_fw(b):
                """Stream in fw rows 16b..16b+16 (fp16)."""
                t = fw_pool.tile([C, 16, HW], f16, tag="fw")
                nc.sync.dma_start(t[:], fw.ap()[:, 16 * b : 16 * b + 16, :])
                fw_tiles[b] = t

            def q_extract(b):
                """Q columns of fw block b: rows 4i+1,4i+2, cols 4j+1,4j+2.
                On GpSimd (no PSUM need, keeps DVE free)."""
                t = fw_tiles[b]
                nbq, half = divmod(b, 2)
                if half == 0:
                    q_tiles[nbq] = qb_pool.tile([C, 512], bf16, tag="q", name="qt")
                qt = q_tiles[nbq]
                fw4 = t[:].rearrange("c (j r) w -> c j r w", r=4)
                rsum = rs_pool.tile([C, 4, 256], bf16, tag="qrs")
                nc.gpsimd.tensor_tensor(
                    rsum[:], fw4[:, :, 1, :], fw4[:, :, 2, :], AOp.add
                )
                rs4 = rsum[:].rearrange("c j (k f) -> c j k f", f=4)
                qv = qt[:, half * 256 : half * 256 + 256].rearrange(
                    "c (j k) -> c j k", k=hw
                )
                nc.gpsimd.tensor_tensor(qv, rs4[:, :, :, 1], rs4[:, :, :, 2], AOp.add)

            def load_k_chunk(cc):
                """fn rows 4i+1,4i+2 for i=4cc..4cc+4 -> k_bf cols (DVE sums).
                kt1 transposes/casts are emitted later, inside iteration 0,
                so they don't head-block the PE/ScalarE FIFOs."""
                ch = kch_pool.tile([C, 4, 512], f16, tag="kch")
                nc.sync.dma_start(ch[:], fn3[:, 4 * cc : 4 * cc + 4, 256:768])
                ch2 = ch[:].rearrange("c i (r w) -> c i r w", r=2)
                rsum = rs_pool.tile([C, 4, 256], bf16, tag="krs")
                nc.vector.tensor_tensor(rsum[:], ch2[:, :, 0, :], ch2[:, :, 1, :], AOp.add)
                rs4 = rsum[:].rearrange("c j (k f) -> c j k f", f=4)
                kv = k_bf[:, 256 * cc : 256 * cc + 256].rearrange(
                    "c (j k) -> c j k", k=hw
                )
                nc.vector.tensor_tensor(kv, rs4[:, :, :, 1], rs4[:, :, :, 2], AOp.add)

            def make_kt1(cc):
                """Transpose chunk cc's k_bf tiles into kt1 (PE + ScalarE)."""
                for mi in (2 * cc, 2 * cc + 1):
                    pt = ps_b.tile([128, 512], f32, tag="bd")
                    ptb = pt[:, 0:64].bitcast(bf16)  # [128, 128] bf16 view
                    nc.tensor.transpose(
                        ptb, k_bf[:, 128 * mi : 128 * mi + 128], id_t[:]
                    )
                    nc.scalar.copy(kt1[:, mi, 0:128], ptb)

            # ---- band machinery ----
            def band_source(m):
                """(kind, t, matrix name) for band m; kind 'ot' or 'z'."""
                if m == 0:
                    return ("ot", 0, "ET")
                if m == 127:
                    return ("ot", 31, "EB")
                if m % 2 == 0:
                    k = m // 2
                    if k % 2 == 1:
                        return ("ot", (k - 1) // 2, "IN0")
                    return ("z", k // 2 - 1, "ZE")
                k = (m - 1) // 2
                if k % 2 == 0:
                    return ("ot", k // 2, "IN1")
                return ("z", (k - 1) // 2, "ZO")

            def band_stationary(m):
                kind, t, name = band_source(m)
                if kind == "ot":
                    return ot_tiles[t // 4][:, t % 4, :], name
                return zt_tiles[(t + 1) // 4][:, (t + 1) % 4, :], name

            def emit_band(m):
                """Band m = output rows (2m, 2m+1): one PE matmul into PSUM,
                DVE add of fw rows in place (fp16), write block when done."""
                st, name = band_stationary(m)
                bd = ps_b.tile([128, 512], f32, tag="bd")
                nc.tensor.matmul(
                    bd[:], st, bm_t[:, MIDX[name], :], start=True, stop=True
                )
                b = m // 8
                lr = 2 * (m % 8)
                fwb = fw_tiles[b]
                nc.vector.tensor_tensor(
                    fwb[:, lr : lr + 2, :], bd[:], fwb[:, lr : lr + 2, :], AOp.add
                )
                if m % 8 == 7:
                    nc.sync.dma_start(
                        out.ap()[:, 16 * b : 16 * b + 16, :], fw_tiles.pop(b)[:]
                    )

            def make_norms(nb, pos):
                """Normalize the 4 sub-tiles of block nb -> ot tile."""
                otb = ot_pool.tile([128, 4, 128], bf16, tag="ot", name="ot_t")
                ot_tiles[nb] = otb
                for ns in range(4):
                    po = pos[ns]
                    rcp = sm_pool.tile([128, 1], f32, tag="rcp")
                    nc.vector.reciprocal(rcp[:], po[:, 128:129])
                    nc.vector.tensor_scalar(
                        otb[:, ns, :], po[:, 0:128], rcp[:], None, AOp.mult
                    )

            def make_z(nb):
                """Z tiles for block nb: slot s = Z_{4nb-1+s} =
                [y[2t+1] ; y[2t+2]] with t = 4nb-1+s.  SBUF->SBUF DMAs."""
                zb = zt_pool.tile([128, 4, 128], bf16, tag="zt", name="zt_t")
                zt_tiles[nb] = zb
                otb = ot_tiles[nb]
                # upper halves: Z_t[64:128] = ot_{t+1}[0:64] for all 4 slots
                nc.sync.dma_start(zb[64:128, 0:4, :], otb[0:64, 0:4, :])
                # lower halves: Z_t[0:64] = ot_t[64:128]
                nc.sync.dma_start(zb[0:64, 1:4, :], otb[64:128, 0:3, :])
                if nb > 0:
                    nc.sync.dma_start(
                        zb[0:64, 0, :], ot_tiles[nb - 1][64:128, 3, :]
                    )

            # ---- prologue ----
            # Dispatch order on sync tuned for the iteration-0 critical path:
            # kch0/1 gate the first transposes, fw0/1 gate qt(0), bm_t is
            # only needed by the first bands (~iteration 1), fw2/3 only by
            # qx late in iteration 0.
            load_k_chunk(0)
            load_k_chunk(1)
            load_fw(0)
            load_fw(1)
            load_k_chunk(2)
            load_k_chunk(3)
            for j in range(6):
                nc.sync.dma_start(bm_t[:, j, :], bm.ap()[j, :, :])
            for cc in range(4, 12):
                load_k_chunk(cc)
            load_fw(2)
            load_fw(3)
            for cc in range(12, 16):
                load_k_chunk(cc)
            q_extract(0)
            q_extract(1)

            bandq = []

            def band_step():
                if bandq:
                    emit_band(bandq.pop(0))

            # ---- main loop ----
            for nb in range(NBLK):
                po0 = ps_o.tile([128, 2, 129], f32, tag="po")
                po1 = ps_o.tile([128, 2, 129], f32, tag="po")
                pos = (po0[:, 0, :], po0[:, 1, :], po1[:, 0, :], po1[:, 1, :])
                qt = q_tiles[nb]

                def spair(mp):
                    ps = ps_s.tile([128, 2, 512], f32, tag="ps")
                    for s in range(2):
                        mi = 2 * mp + s
                        nc.tensor.matmul(
                            ps[:, s, :],
                            k_bf[:, 128 * mi : 128 * mi + 128],
                            qt[:],
                            start=True,
                            stop=True,
                        )
                    nc.scalar.activation(
                        at[:, 2 * mp : 2 * mp + 2, :],
                        ps[:],
                        ActFn.Exp,
                        bias=0.0,
                        scale=EXP_SCALE,
                    )

                def pv_chunk(mp):
                    for ns in range(4):
                        for mi in (2 * mp, 2 * mp + 1):
                            nc.tensor.matmul(
                                pos[ns],
                                at[:, mi, 128 * ns : 128 * ns + 128],
                                kt1[:, mi, :],
                                start=(mi == 0),
                                stop=(mi == MI - 1),
                            )

                qx_slots = (12, 15) if nb == 0 else (5, 11)
                for mp in range(16):
                    if nb == 0:
                        make_kt1(mp)
                    spair(mp)
                    if mp >= 2:
                        pv_chunk(mp - 2)
                    if nb < NBLK - 1 or mp >= 3:
                        band_step()
                    if mp == 0:
                        if 2 * nb + 4 < NFB:
                            load_fw(2 * nb + 4)
                    if mp == qx_slots[0]:
                        if 2 * nb + 2 < NFB:
                            q_extract(2 * nb + 2)
                    if mp == 10:
                        if 2 * nb + 5 < NFB:
                            load_fw(2 * nb + 5)
                    if mp == qx_slots[1]:
                        if 2 * nb + 3 < NFB:
                            q_extract(2 * nb + 3)
                pv_chunk(14)
                if nb < NBLK - 1:
                    band_step()
                pv_chunk(15)
                if nb < NBLK - 1:
                    band_step()
                make_norms(nb, pos)
                make_z(nb)
                # newly enabled bands: contiguous run 16nb-1 .. 16nb+14.
                # The two bands on Z_{4nb-1} (16nb-1, 16nb) are moved behind
                # the first two direct bands so they don't race the Z DMA.
                # The last run (nb=7) is handled by the epilogue instead.
                if nb < NBLK - 1:
                    lo = max(0, 16 * nb - 1)
                    run = list(range(lo, 16 * nb + 15))
                    if nb > 0:
                        run = [run[2], run[3], run[0], run[1]] + run[4:]
                    bandq.extend(run)
            # held-back run-6 bands fill the PE while the last norms and Z
            # tiles are produced
            while bandq:
                emit_band(bandq.pop(0))

            # ---- epilogue: bands 111..127 (tiles 28..31 & Z 27..30).
            # Scores are done, so ps_s banks are free: pair bands into
            # [128,2,512] tiles -> one DVE add per 4 output rows.  The last
            # two blocks are written out in halves so the final DMA overlaps
            # the remaining adds.
            emit_band(111)  # completes block 13
            for m0 in range(112, 128, 2):
                bd2 = ps_s.tile([128, 2, 512], f32, tag="ps")
                for j, m in enumerate((m0, m0 + 1)):
                    st, name = band_stationary(m)
                    nc.tensor.matmul(
                        bd2[:, j, :], st, bm_t[:, MIDX[name], :],
                        start=True, stop=True,
                    )
                b = m0 // 8
                lr = 2 * (m0 % 8)
                fwb = fw_tiles[b]
                nc.vector.tensor_tensor(
                    fwb[:, lr : lr + 4, :], bd2[:], fwb[:, lr : lr + 4, :], AOp.add
                )
                if (m0 + 1) % 8 == 3:
                    nc.sync.dma_start(
                        out.ap()[:, 16 * b : 16 * b + 8, :], fwb[:, 0:8, :]
                    )
                elif (m0 + 1) % 8 == 7:
                    nc.sync.dma_start(
                        out.ap()[:, 16 * b + 8 : 16 * b + 16, :],
                        fw_tiles.pop(b)[:, 8:16, :],
                    )

    nc.compile()
    return nc


_NC_CACHE = None


def _get_nc():
    global _NC_CACHE
    if _NC_CACHE is None:
        _NC_CACHE = build_kernel()
    return _NC_CACHE


def run(feat_wide: np.ndarray, feat_narrow: np.ndarray, trace: bool = False):
    """Run on 8 NeuronCores; returns (output [8,128,256,256], BassKernelResults)."""
    from concourse.bass_utils import run_bass_kernel_spmd
    import ml_dtypes

    B, C, H, W = feat_wide.shape
    assert (B, C, H, W) == (8, 128, 256, 256)

    bandmat = _build_band_matrices().astype(ml_dtypes.bfloat16)
    identity = np.eye(128, dtype=ml_dtypes.bfloat16)

    fw16 = np.asarray(feat_wide, dtype=np.float16)
    fn16 = np.asarray(feat_narrow, dtype=np.float16)

    nc = _get_nc()
    in_maps = [
        {
            "feat_wide": np.ascontiguousarray(fw16[b]),
            "feat_narrow": np.ascontiguousarray(fn16[b]),
            "bandmat": bandmat,
            "ident": identity,
        }
        for b in range(B)
    ]
    res = run_bass_kernel_spmd(nc, in_maps, core_ids=list(range(8)), trace=trace)
    outv = np.stack(
        [np.asarray(res.results[b]["out"], dtype=np.float32) for b in range(B)],
        axis=0,
    )
    return outv, res


def kernel(feat_wide: np.ndarray, feat_narrow: np.ndarray) -> np.ndarray:
    out, _ = run(feat_wide, feat_narrow, trace=False)
    return out


# revision 17
# speedup vs baseline: 1.5173x; 1.0241x over previous
"""Trainium2 Bass kernel v5 for nn_AttentionFusion — fp16 staging, fused bands,
in-iteration PV.

Per batch element (B=8 -> one NeuronCore each):
    Q = ds(feat_wide), K = ds(feat_narrow)      # 2x2 center sums, [C, 4096]
    attn = softmax(Q^T K / (16 sqrt(C)))
    out = feat_wide + up4((attn @ K^T)^T)       # bilinear 4x upsample + residual

v5 changes vs v4:
  - fw / fn / out staged in DRAM as fp16 (host casts): HBM traffic 85 -> 42 MB.
  - Cross-tile bands fused: partition-shifted Z tiles (built by SBUF->SBUF
    DMA after the norms) let every band be a single matmul (128 total).
  - PV(nb) runs IN iteration nb, trailing exp by 2 m-pair slots; norms at
    iteration end; bands of the just-normalized tiles run next iteration.
    Pipeline depth ~1 iteration; short epilogue.
  - PE warmup matmuls at t~0 so the HAM clock gate is at 8/8 when scores
    start; kt1 PSUM->SBUF casts on ScalarE (idle during prologue).

Engine mapping steady state (per n-block iteration ~21 us):
  - PE: 32 score mms + 128 PV mms + 16 band mms (+ kt1 transposes, iter 0)
  - ScalarE: exp per m-pair (PSUM -> fp8 at)
  - DVE: band adds (PSUM + fw fp16 -> fp16), norms, k-chunk sums (prologue)
  - GpSimd: Q extraction (fp16 strided sums)
  - Sync: all DMA dispatch (loads, Z copies, out writes)
"""

import math

import numpy as np


# ----------------------------------------------------------------------------
# numpy-side constants
# ----------------------------------------------------------------------------

def _build_upsample_matrix(n_in: int, n_out: int) -> np.ndarray:
    """U[h, H]: out[H] = sum_h U[h, H] * in[h] for torch-style bilinear,
    align_corners=False, antialias=False, scale n_out/n_in."""
    U = np.zeros((n_in, n_out), dtype=np.float64)
    scale = n_in / n_out
    for o in range(n_out):
        src = (o + 0.5) * scale - 0.5
        k0 = int(math.floor(src))
        frac = src - k0
        for k, wt in ((k0, 1.0 - frac), (k0 + 1, frac)):
            kc = min(max(k, 0), n_in - 1)
            U[kc, o] += wt
    return U


# Band m covers output rows (2m, 2m+1).  With y[k] the H-downsampled rows,
# H-up row 4k+r draws from y via 2 taps.  ot tile t holds y rows (2t, 2t+1)
# as partitions (h'=0/1) x 64 w; Z tile t holds [ot_t h'=1 ; ot_{t+1} h'=0]
# so every band is ONE matmul:
#   m=4t+1: IN1 on ot_t     rows (4k+2,4k+3), k=2t   even
#   m=4t+2: IN0 on ot_t     rows (4k,  4k+1), k=2t+1 odd
#   m=4t+3: ZO  on Z_t      rows (4k+2,4k+3), k=2t+1 odd
#   m=4t+4: ZE  on Z_t      rows (4k,  4k+1), k=2t+2 even
#   m=0: ET on ot_0; m=127: EB on ot_31 (edge clamps)
# Matrix[(h'',w),(r2,W)] = coef[h''][r2] * Uw[w,W], Uw = 0.25*U(64->256)
# (0.25 undoes the unscaled 2x2-sum downsample of K).
_BAND_COEFS = {
    "IN0": {0: (0.375, 0.125), 1: (0.625, 0.875)},  # h'0=y[k-1], h'1=y[k]
    "IN1": {0: (0.875, 0.625), 1: (0.125, 0.375)},  # h'0=y[k],   h'1=y[k+1]
    "ZE": {0: (0.375, 0.125), 1: (0.625, 0.875)},   # low=y[k-1], up=y[k]
    "ZO": {0: (0.875, 0.625), 1: (0.125, 0.375)},   # low=y[k],   up=y[k+1]
    "ET": {0: (1.0, 1.0)},                          # rows 0,1 clamp to y[0]
    "EB": {1: (1.0, 1.0)},                          # rows 254,255 -> y[63]
}
_BAND_NAMES = ("IN0", "IN1", "ZE", "ZO", "ET", "EB")


def _build_band_matrices() -> np.ndarray:
    """[6, 128, 512] f64: fused H-up x W-up band weight matrices."""
    Uw = _build_upsample_matrix(64, 256) * 0.25  # [64, 256]
    out = np.zeros((6, 128, 512), dtype=np.float64)
    for idx, name in enumerate(_BAND_NAMES):
        for hp, (c0, c1) in _BAND_COEFS[name].items():
            for r2, cf in ((0, c0), (1, c1)):
                out[idx, hp * 64 : hp * 64 + 64, r2 * 256 : r2 * 256 + 256] = (
                    cf * Uw
                )
    return out


# ----------------------------------------------------------------------------
# Bass kernel builder
# ----------------------------------------------------------------------------

def build_kernel():
    import concourse.bacc as bacc
    import concourse.bass as bass
    import concourse.mybir as mybir
    from concourse import tile

    f32 = mybir.dt.float32
    f16 = mybir.dt.float16
    bf16 = mybir.dt.bfloat16
    f8 = mybir.dt.float8e4
    AOp = mybir.AluOpType
    ActFn = mybir.ActivationFunctionType

    C = 128          # channels = partitions
    HW = 256         # full resolution
    hw = 64          # downsampled resolution
    N = hw * hw      # 4096 attention positions
    MI = 32          # m tiles of 128
    NBLK = 8         # n blocks of 512
    NFB = 16         # fw row blocks of 16 rows
    EXP_SCALE = 1.0 / (16.0 * math.sqrt(C))
    MIDX = {n: i for i, n in enumerate(_BAND_NAMES)}

    nc = bacc.Bacc("TRN2", target_bir_lowering=False, debug=False)

    fw = nc.dram_tensor("feat_wide", [C, HW, HW], f16, kind="ExternalInput")
    fn = nc.dram_tensor("feat_narrow", [C, HW, HW], f16, kind="ExternalInput")
    bm = nc.dram_tensor("bandmat", [6, 128, 512], bf16, kind="ExternalInput")
    ident = nc.dram_tensor("ident", [128, 128], bf16, kind="ExternalInput")
    out = nc.dram_tensor("out", [C, HW, HW], f16, kind="ExternalOutput")

    with tile.TileContext(nc) as tc:
        with (
            tc.tile_pool(name="const", bufs=1) as const_pool,
            tc.tile_pool(name="big", bufs=1) as big_pool,
            tc.tile_pool(name="fwp", bufs=8) as fw_pool,
            tc.tile_pool(name="kch", bufs=4) as kch_pool,
            tc.tile_pool(name="qb", bufs=2) as qb_pool,
            tc.tile_pool(name="rs", bufs=2) as rs_pool,
            tc.tile_pool(name="ot", bufs=2) as ot_pool,
            tc.tile_pool(name="zt", bufs=2) as zt_pool,
            tc.tile_pool(name="sm", bufs=4) as sm_pool,
            tc.tile_pool(name="ps_s", bufs=2, space=bass.MemorySpace.PSUM) as ps_s,
            tc.tile_pool(name="ps_o", bufs=2, space=bass.MemorySpace.PSUM) as ps_o,
            tc.tile_pool(name="ps_b", bufs=2, space=bass.MemorySpace.PSUM) as ps_b,
        ):
            # ---- constants (ident first: it gates the PE warmup) ----
            id_t = const_pool.tile([128, 128], bf16)
            nc.sync.dma_start(id_t[:], ident[:, :])
            bm_t = const_pool.tile([128, 6, 512], bf16)

            # ---- persistent buffers ----
            k_bf = big_pool.tile([C, N], bf16)
            kt1 = big_pool.tile([128, MI, 129], f8)   # K^T tiles + ones col
            nc.vector.memset(kt1[:], 1.0)
            at = big_pool.tile([128, MI, 512], f8)    # attn^T (exp'd scores)

            fn3 = fn.ap().rearrange("c (i r) w -> c i (r w)", r=4)

            fw_tiles = {}
            q_tiles = {}
            ot_tiles = {}   # nb -> [128, 4, 128] tile (y sub-tiles of block nb)
            zt_tiles = {}   # nb -> [128, 4, 128] tile (slot s = Z_{4nb-1+s})

            # ---- PE warmup: keep the HAM clock-gate busy from t~0 so the
            # first real matmuls run at 2.4 GHz.  Junk mms on the identity.
            pwarm = ps_b.tile([128, 512], f32, tag="bd")
            for _ in range(20):
                nc.tensor.matmul(
                    pwarm[:, 0:128], id_t[:], id_t[:], start=True, stop=True
                )

            def load_fw(b):
                """Stream in fw rows 16b..16b+16 (fp16)."""
                t = fw_pool.tile([C, 16, HW], f16, tag="fw")
                nc.sync.dma_start(t[:], fw.ap()[:, 16 * b : 16 * b + 16, :])
                fw_tiles[b] = t

            def q_extract(b, eng):
                """Q columns of fw block b: rows 4i+1,4i+2, cols 4j+1,4j+2."""
                t = fw_tiles[b]
                nbq, half = divmod(b, 2)
                if half == 0:
                    q_tiles[nbq] = qb_pool.tile([C, 512], bf16, tag="q", name="qt")
                qt = q_tiles[nbq]
                fw4 = t[:].rearrange("c (j r) w -> c j r w", r=4)
                rsum = rs_pool.tile([C, 4, 256], bf16, tag="qrs")
                eng.tensor_tensor(
                    rsum[:], fw4[:, :, 1, :], fw4[:, :, 2, :], AOp.add
                )
                rs4 = rsum[:].rearrange("c j (k f) -> c j k f", f=4)
                qv = qt[:, half * 256 : half * 256 + 256].rearrange(
                    "c (j k) -> c j k", k=hw
                )
                eng.tensor_tensor(qv, rs4[:, :, :, 1], rs4[:, :, :, 2], AOp.add)

            def load_k_chunk(cc):
                """fn rows 4i+1,4i+2 for i=4cc..4cc+4 -> k_bf cols (DVE sums).
                kt1 transposes/casts are emitted later, inside iteration 0,
                so they don't head-block the PE/ScalarE FIFOs."""
                ch = kch_pool.tile([C, 4, 512], f16, tag="kch")
                nc.sync.dma_start(ch[:], fn3[:, 4 * cc : 4 * cc + 4, 256:768])
                ch2 = ch[:].rearrange("c i (r w) -> c i r w", r=2)
                rsum = rs_pool.tile([C, 4, 256], bf16, tag="krs")
                nc.vector.tensor_tensor(rsum[:], ch2[:, :, 0, :], ch2[:, :, 1, :], AOp.add)
                rs4 = rsum[:].rearrange("c j (k f) -> c j k f", f=4)
                kv = k_bf[:, 256 * cc : 256 * cc + 256].rearrange(
                    "c (j k) -> c j k", k=hw
                )
                nc.vector.tensor_tensor(kv, rs4[:, :, :, 1], rs4[:, :, :, 2], AOp.add)

            def make_kt1(cc):
                """Transpose chunk cc's k_bf tiles into kt1 (PE + ScalarE)."""
                for mi in (2 * cc, 2 * cc + 1):
                    pt = ps_b.tile([128, 512], f32, tag="bd")
                    ptb = pt[:, 0:64].bitcast(bf16)  # [128, 128] bf16 view
                    nc.tensor.transpose(
                        ptb, k_bf[:, 128 * mi : 128 * mi + 128], id_t[:]
                    )
                    nc.scalar.copy(kt1[:, mi, 0:128], ptb)

            # ---- band machinery ----
            def band_source(m):
                """(kind, t, matrix name) for band m; kind 'ot' or 'z'."""
                if m == 0:
                    return ("ot", 0, "ET")
                if m == 127:
                    return ("ot", 31, "EB")
                if m % 2 == 0:
                    k = m // 2
                    if k % 2 == 1:
                        return ("ot", (k - 1) // 2, "IN0")
                    return ("z", k // 2 - 1, "ZE")
                k = (m - 1) // 2
                if k % 2 == 0:
                    return ("ot", k // 2, "IN1")
                return ("z", (k - 1) // 2, "ZO")

            def band_stationary(m):
                kind, t, name = band_source(m)
                if kind == "ot":
                    return ot_tiles[t // 4][:, t % 4, :], name
                return zt_tiles[(t + 1) // 4][:, (t + 1) % 4, :], name

            def emit_band(m):
                """Band m = output rows (2m, 2m+1): one PE matmul into PSUM,
                DVE add of fw rows in place (fp16), write block when done."""
                st, name = band_stationary(m)
                bd = ps_b.tile([128, 512], f32, tag="bd")
                nc.tensor.matmul(
                    bd[:], st, bm_t[:, MIDX[name], :], start=True, stop=True
                )
                b = m // 8
                lr = 2 * (m % 8)
                fwb = fw_tiles[b]
                nc.vector.tensor_tensor(
                    fwb[:, lr : lr + 2, :], bd[:], fwb[:, lr : lr + 2, :], AOp.add
                )
                if m % 8 == 7:
                    nc.sync.dma_start(
                        out.ap()[:, 16 * b : 16 * b + 16, :], fw_tiles.pop(b)[:]
                    )

            def make_norms(nb, pos):
                """Normalize the 4 sub-tiles of block nb -> ot tile."""
                otb = ot_pool.tile([128, 4, 128], bf16, tag="ot", name="ot_t")
                ot_tiles[nb] = otb
                for ns in range(4):
                    po = pos[ns]
                    rcp = sm_pool.tile([128, 1], f32, tag="rcp")
                    nc.vector.reciprocal(rcp[:], po[:, 128:129])
                    nc.vector.tensor_scalar(
                        otb[:, ns, :], po[:, 0:128], rcp[:], None, AOp.mult
                    )

            def make_z(nb):
                """Z tiles for block nb: slot s = Z_{4nb-1+s} =
                [y[2t+1] ; y[2t+2]] with t = 4nb-1+s.  SBUF->SBUF DMAs."""
                zb = zt_pool.tile([128, 4, 128], bf16, tag="zt", name="zt_t")
                zt_tiles[nb] = zb
                otb = ot_tiles[nb]
                # upper halves: Z_t[64:128] = ot_{t+1}[0:64] for all 4 slots
                nc.sync.dma_start(zb[64:128, 0:4, :], otb[0:64, 0:4, :])
                # lower halves: Z_t[0:64] = ot_t[64:128]
                nc.sync.dma_start(zb[0:64, 1:4, :], otb[64:128, 0:3, :])
                if nb > 0:
                    nc.sync.dma_start(
                        zb[0:64, 0, :], ot_tiles[nb - 1][64:128, 3, :]
                    )

            # ---- prologue ----
            # Dispatch order on sync tuned for the iteration-0 critical path:
            # kch0/1 gate the first transposes, fw0/1 gate qt(0), bm_t is
            # only needed by the first bands (~iteration 1), fw2/3 only by
            # qx late in iteration 0.
            load_k_chunk(0)
            load_k_chunk(1)
            load_fw(0)
            load_fw(1)
            q_extract(0, nc.gpsimd)   # waits fw0 on gpsimd
            q_extract(1, nc.vector)   # on DVE right behind kch0/1 sums
            load_k_chunk(2)
            load_k_chunk(3)
            for j in range(6):
                nc.sync.dma_start(bm_t[:, j, :], bm.ap()[j, :, :])
            for cc in range(4, 9):
                load_k_chunk(cc)
            load_fw(2)
            load_fw(3)
            for cc in range(9, 16):
                load_k_chunk(cc)

            bandq = []

            def band_step():
                if bandq:
                    emit_band(bandq.pop(0))

            pos_of = {}

            def spair(g):
                """Scores m-pair: block g//16, pair g%16; exp -> at."""
                nbp, mp = divmod(g, 16)
                qt = q_tiles[nbp]
                ps = ps_s.tile([128, 2, 512], f32, tag="ps")
                for s in range(2):
                    mi = 2 * mp + s
                    nc.tensor.matmul(
                        ps[:, s, :],
                        k_bf[:, 128 * mi : 128 * mi + 128],
                        qt[:],
                        start=True,
                        stop=True,
                    )
                nc.scalar.activation(
                    at[:, 2 * mp : 2 * mp + 2, :],
                    ps[:],
                    ActFn.Exp,
                    bias=0.0,
                    scale=EXP_SCALE,
                )

            def pv_chunk(p):
                """PV accumulation for global pair p (block p//16)."""
                nbp, mp = divmod(p, 16)
                if mp == 0:
                    po0 = ps_o.tile([128, 2, 129], f32, tag="po")
                    po1 = ps_o.tile([128, 2, 129], f32, tag="po")
                    pos_of[nbp] = (
                        po0[:, 0, :], po0[:, 1, :], po1[:, 0, :], po1[:, 1, :]
                    )
                pos = pos_of[nbp]
                for ns in range(4):
                    for mi in (2 * mp, 2 * mp + 1):
                        nc.tensor.matmul(
                            pos[ns],
                            at[:, mi, 128 * ns : 128 * ns + 128],
                            kt1[:, mi, :],
                            start=(mi == 0),
                            stop=(mi == MI - 1),
                        )

            def finish_block(nbf):
                """Norms + Z tiles + enable the band run for block nbf."""
                make_norms(nbf, pos_of.pop(nbf))
                make_z(nbf)
                # run 16nbf-1 .. 16nbf+14; the two bands on Z_{4nbf-1}
                # (16nbf-1, 16nbf) go behind the first two direct bands so
                # they don't race the Z DMA.  Run 7 is the epilogue's.
                if nbf < NBLK - 1:
                    lo = max(0, 16 * nbf - 1)
                    run = list(range(lo, 16 * nbf + 15))
                    if nbf > 0:
                        run = [run[2], run[3], run[0], run[1]] + run[4:]
                    bandq.extend(run)

            # ---- main loop: one flat 128-slot software pipeline ----
            for g in range(16 * NBLK):
                nb, mp = divmod(g, 16)
                if nb == 0:
                    make_kt1(mp)
                spair(g)
                if g >= 2:
                    pv_chunk(g - 2)
                if mp == 1 and nb >= 1:
                    finish_block(nb - 1)
                if g < 125:
                    band_step()
                if mp == 0 and 2 * nb + 4 < NFB:
                    load_fw(2 * nb + 4)
                if mp == 10 and 2 * nb + 5 < NFB:
                    load_fw(2 * nb + 5)
                qx_slots = (10, 13) if nb == 0 else (5, 11)
                if mp == qx_slots[0] and 2 * nb + 2 < NFB:
                    q_extract(2 * nb + 2, nc.gpsimd)
                if mp == qx_slots[1] and 2 * nb + 3 < NFB:
                    q_extract(2 * nb + 3, nc.vector if nb == 0 else nc.gpsimd)
            pv_chunk(126)
            pv_chunk(127)
            finish_block(NBLK - 1)
            # held-back bands fill the PE while the last norms and Z tiles
            # are produced
            while bandq:
                emit_band(bandq.pop(0))

            # ---- epilogue: bands 111..127 (tiles 28..31 & Z 27..30).
            # Scores are done, so ps_s banks are free: pair bands into
            # [128,2,512] tiles -> one DVE add per 4 output rows.  The last
            # two blocks are written out in halves so the final DMA overlaps
            # the remaining adds.
            emit_band(111)  # completes block 13
            for m0 in range(112, 128, 2):
                bd2 = ps_s.tile([128, 2, 512], f32, tag="ps")
                for j, m in enumerate((m0, m0 + 1)):
                    st, name = band_stationary(m)
                    nc.tensor.matmul(
                        bd2[:, j, :], st, bm_t[:, MIDX[name], :],
                        start=True, stop=True,
                    )
                b = m0 // 8
                lr = 2 * (m0 % 8)
                fwb = fw_tiles[b]
                nc.vector.tensor_tensor(
                    fwb[:, lr : lr + 4, :], bd2[:], fwb[:, lr : lr + 4, :], AOp.add
                )
                if (m0 + 1) % 8 == 3:
                    nc.sync.dma_start(
                        out.ap()[:, 16 * b : 16 * b + 8, :], fwb[:, 0:8, :]
                    )
                elif (m0 + 1) % 8 == 7:
                    nc.sync.dma_start(
                        out.ap()[:, 16 * b + 8 : 16 * b + 16, :],
                        fw_tiles.pop(b)[:, 8:16, :],
                    )

    nc.compile()
    return nc


_NC_CACHE = None


def _get_nc():
    global _NC_CACHE
    if _NC_CACHE is None:
        _NC_CACHE = build_kernel()
    return _NC_CACHE


def run(feat_wide: np.ndarray, feat_narrow: np.ndarray, trace: bool = False):
    """Run on 8 NeuronCores; returns (output [8,128,256,256], BassKernelResults)."""
    from concourse.bass_utils import run_bass_kernel_spmd
    import ml_dtypes

    B, C, H, W = feat_wide.shape
    assert (B, C, H, W) == (8, 128, 256, 256)

    bandmat = _build_band_matrices().astype(ml_dtypes.bfloat16)
    identity = np.eye(128, dtype=ml_dtypes.bfloat16)

    fw16 = np.asarray(feat_wide, dtype=np.float16)
    fn16 = np.asarray(feat_narrow, dtype=np.float16)

    nc = _get_nc()
    in_maps = [
        {
            "feat_wide": np.ascontiguousarray(fw16[b]),
            "feat_narrow": np.ascontiguousarray(fn16[b]),
            "bandmat": bandmat,
            "ident": identity,
        }
        for b in range(B)
    ]
    res = run_bass_kernel_spmd(nc, in_maps, core_ids=list(range(8)), trace=trace)
    outv = np.stack(
        [np.asarray(res.results[b]["out"], dtype=np.float32) for b in range(B)],
        axis=0,
    )
    return outv, res


def kernel(feat_wide: np.ndarray, feat_narrow: np.ndarray) -> np.ndarray:
    out, _ = run(feat_wide, feat_narrow, trace=False)
    return out


# revision 25
# speedup vs baseline: 1.5829x; 1.0433x over previous
"""Trainium2 Bass kernel v5 for nn_AttentionFusion — fp16 staging, fused bands,
in-iteration PV.

Per batch element (B=8 -> one NeuronCore each):
    Q = ds(feat_wide), K = ds(feat_narrow)      # 2x2 center sums, [C, 4096]
    attn = softmax(Q^T K / (16 sqrt(C)))
    out = feat_wide + up4((attn @ K^T)^T)       # bilinear 4x upsample + residual

v5 changes vs v4:
  - fw / fn / out staged in DRAM as fp16 (host casts): HBM traffic 85 -> 42 MB.
  - Cross-tile bands fused: partition-shifted Z tiles (built by SBUF->SBUF
    DMA after the norms) let every band be a single matmul (128 total).
  - PV(nb) runs IN iteration nb, trailing exp by 2 m-pair slots; norms at
    iteration end; bands of the just-normalized tiles run next iteration.
    Pipeline depth ~1 iteration; short epilogue.
  - PE warmup matmuls at t~0 so the HAM clock gate is at 8/8 when scores
    start; kt1 PSUM->SBUF casts on ScalarE (idle during prologue).

Engine mapping steady state (per n-block iteration ~21 us):
  - PE: 32 score mms + 128 PV mms + 16 band mms (+ kt1 transposes, iter 0)
  - ScalarE: exp per m-pair (PSUM -> fp8 at)
  - DVE: band adds (PSUM + fw fp16 -> fp16), norms, k-chunk sums (prologue)
  - GpSimd: Q extraction (fp16 strided sums)
  - Sync: all DMA dispatch (loads, Z copies, out writes)
"""

import math

import numpy as np


# ----------------------------------------------------------------------------
# numpy-side constants
# ----------------------------------------------------------------------------

def _build_upsample_matrix(n_in: int, n_out: int) -> np.ndarray:
    """U[h, H]: out[H] = sum_h U[h, H] * in[h] for torch-style bilinear,
    align_corners=False, antialias=False, scale n_out/n_in."""
    U = np.zeros((n_in, n_out), dtype=np.float64)
    scale = n_in / n_out
    for o in range(n_out):
        src = (o + 0.5) * scale - 0.5
        k0 = int(math.floor(src))
        frac = src - k0
        for k, wt in ((k0, 1.0 - frac), (k0 + 1, frac)):
            kc = min(max(k, 0), n_in - 1)
            U[kc, o] += wt
    return U


# Band m covers output rows (2m, 2m+1).  With y[k] the H-downsampled rows,
# H-up row 4k+r draws from y via 2 taps.  ot tile t holds y rows (2t, 2t+1)
# as partitions (h'=0/1) x 64 w; Z tile t holds [ot_t h'=1 ; ot_{t+1} h'=0]
# so every band is ONE matmul:
#   m=4t+1: IN1 on ot_t     rows (4k+2,4k+3), k=2t   even
#   m=4t+2: IN0 on ot_t     rows (4k,  4k+1), k=2t+1 odd
#   m=4t+3: ZO  on Z_t      rows (4k+2,4k+3), k=2t+1 odd
#   m=4t+4: ZE  on Z_t      rows (4k,  4k+1), k=2t+2 even
#   m=0: ET on ot_0; m=127: EB on ot_31 (edge clamps)
# Matrix[(h'',w),(r2,W)] = coef[h''][r2] * Uw[w,W], Uw = 0.25*U(64->256)
# (0.25 undoes the unscaled 2x2-sum downsample of K).
_BAND_COEFS = {
    "IN0": {0: (0.375, 0.125), 1: (0.625, 0.875)},  # h'0=y[k-1], h'1=y[k]
    "IN1": {0: (0.875, 0.625), 1: (0.125, 0.375)},  # h'0=y[k],   h'1=y[k+1]
    "ZE": {0: (0.375, 0.125), 1: (0.625, 0.875)},   # low=y[k-1], up=y[k]
    "ZO": {0: (0.875, 0.625), 1: (0.125, 0.375)},   # low=y[k],   up=y[k+1]
    "ET": {0: (1.0, 1.0)},                          # rows 0,1 clamp to y[0]
    "EB": {1: (1.0, 1.0)},                          # rows 254,255 -> y[63]
}
_BAND_NAMES = ("IN0", "IN1", "ZE", "ZO", "ET", "EB")


def _build_band_matrices() -> np.ndarray:
    """[6, 128, 512] f64: fused H-up x W-up band weight matrices."""
    Uw = _build_upsample_matrix(64, 256) * 0.25  # [64, 256]
    out = np.zeros((6, 128, 512), dtype=np.float64)
    for idx, name in enumerate(_BAND_NAMES):
        for hp, (c0, c1) in _BAND_COEFS[name].items():
            for r2, cf in ((0, c0), (1, c1)):
                out[idx, hp * 64 : hp * 64 + 64, r2 * 256 : r2 * 256 + 256] = (
                    cf * Uw
                )
    return out


# ----------------------------------------------------------------------------
# Bass kernel builder
# ----------------------------------------------------------------------------

def build_kernel():
    import concourse.bacc as bacc
    import concourse.bass as bass
    import concourse.mybir as mybir
    from concourse import tile

    f32 = mybir.dt.float32
    f16 = mybir.dt.float16
    bf16 = mybir.dt.bfloat16
    f8 = mybir.dt.float8e4
    AOp = mybir.AluOpType
    ActFn = mybir.ActivationFunctionType

    C = 128          # channels = partitions
    HW = 256         # full resolution
    hw = 64          # downsampled resolution
    N = hw * hw      # 4096 attention positions
    MI = 32          # m tiles of 128
    NBLK = 8         # n blocks of 512
    NFB = 16         # fw row blocks of 16 rows
    EXP_SCALE = 1.0 / (16.0 * math.sqrt(C))
    MIDX = {n: i for i, n in enumerate(_BAND_NAMES)}

    nc = bacc.Bacc("TRN2", target_bir_lowering=False, debug=False)

    fw = nc.dram_tensor("feat_wide", [C, HW, HW], f16, kind="ExternalInput")
    fn = nc.dram_tensor("feat_narrow", [C, HW, HW], f16, kind="ExternalInput")
    bm = nc.dram_tensor("bandmat", [6, 128, 512], bf16, kind="ExternalInput")
    ident = nc.dram_tensor("ident", [128, 128], bf16, kind="ExternalInput")
    out = nc.dram_tensor("out", [C, HW, HW], f16, kind="ExternalOutput")

    with tile.TileContext(nc) as tc:
        with (
            tc.tile_pool(name="const", bufs=1) as const_pool,
            tc.tile_pool(name="big", bufs=1) as big_pool,
            tc.tile_pool(name="fwp", bufs=8) as fw_pool,
            tc.tile_pool(name="kch", bufs=4) as kch_pool,
            tc.tile_pool(name="wqp", bufs=4) as wq_pool,
            tc.tile_pool(name="qb", bufs=2) as qb_pool,
            tc.tile_pool(name="rs", bufs=2) as rs_pool,
            tc.tile_pool(name="ot", bufs=2) as ot_pool,
            tc.tile_pool(name="zt", bufs=2) as zt_pool,
            tc.tile_pool(name="sm", bufs=4) as sm_pool,
            tc.tile_pool(name="ps_s", bufs=2, space=bass.MemorySpace.PSUM) as ps_s,
            tc.tile_pool(name="ps_o", bufs=2, space=bass.MemorySpace.PSUM) as ps_o,
            tc.tile_pool(name="ps_b", bufs=2, space=bass.MemorySpace.PSUM) as ps_b,
        ):
            # ---- constants (ident first: it gates the PE warmup) ----
            id_t = const_pool.tile([128, 128], bf16)
            nc.sync.dma_start(id_t[:], ident[:, :])
            bm_t = const_pool.tile([128, 6, 512], bf16)

            # ---- persistent buffers ----
            k_bf = big_pool.tile([C, N], bf16)
            kt1 = big_pool.tile([128, MI, 129], f8)   # K^T tiles + ones col
            nc.vector.memset(kt1[:], 1.0)
            at = big_pool.tile([128, MI, 512], f8)    # attn^T (exp'd scores)

            fn3 = fn.ap().rearrange("c (i r) w -> c i (r w)", r=4)
            fwv = fw.ap().rearrange("c (i r) w -> c i (r w)", r=4)

            fw_tiles = {}
            q_tiles = {}
            ot_tiles = {}   # nb -> [128, 4, 128] tile (y sub-tiles of block nb)
            zt_tiles = {}   # nb -> [128, 4, 128] tile (slot s = Z_{4nb-1+s})

            # ---- PE warmup: keep the HAM clock-gate busy from t~0 so the
            # first real matmuls run at 2.4 GHz.  Junk mms on the identity.
            pwarm = ps_b.tile([128, 512], f32, tag="bd")
            for _ in range(36):
                nc.tensor.matmul(
                    pwarm[:, 0:128], id_t[:], id_t[:], start=True, stop=True
                )

            def load_fw(b):
                """Stream in fw rows 16b..16b+16 (fp16)."""
                t = fw_pool.tile([C, 16, HW], f16, tag="fw")
                nc.sync.dma_start(t[:], fw.ap()[:, 16 * b : 16 * b + 16, :])
                fw_tiles[b] = t

            def _q_finish(b, eng, rsum):
                nbq, half = divmod(b, 2)
                if half == 0:
                    q_tiles[nbq] = qb_pool.tile([C, 512], bf16, tag="q", name="qt")
                qt = q_tiles[nbq]
                rs4 = rsum[:].rearrange("c j (k f) -> c j k f", f=4)
                qv = qt[:, half * 256 : half * 256 + 256].rearrange(
                    "c (j k) -> c j k", k=hw
                )
                eng.tensor_tensor(qv, rs4[:, :, :, 1], rs4[:, :, :, 2], AOp.add)

            def q_extract(b, eng):
                """Q columns of fw block b: rows 4i+1,4i+2, cols 4j+1,4j+2."""
                t = fw_tiles[b]
                fw4 = t[:].rearrange("c (j r) w -> c j r w", r=4)
                rsum = rs_pool.tile([C, 4, 256], bf16, tag="qrs")
                eng.tensor_tensor(
                    rsum[:], fw4[:, :, 1, :], fw4[:, :, 2, :], AOp.add
                )
                _q_finish(b, eng, rsum)

            wq_tiles = {}

            def load_wq(b):
                """Compact Q-row stream for an early block: rows 4i+1,4i+2
                only (0.5 MB) so qt doesn't wait for the full fw block."""
                t = wq_pool.tile([C, 4, 512], f16, tag="wq")
                nc.sync.dma_start(t[:], fwv[:, 4 * b : 4 * b + 4, 256:768])
                wq_tiles[b] = t

            def q_extract_wq(b, eng):
                w2 = wq_tiles.pop(b)[:].rearrange("c i (r w) -> c i r w", r=2)
                rsum = rs_pool.tile([C, 4, 256], bf16, tag="qrs")
                eng.tensor_tensor(rsum[:], w2[:, :, 0, :], w2[:, :, 1, :], AOp.add)
                _q_finish(b, eng, rsum)

            def load_k_chunk(cc):
                """fn rows 4i+1,4i+2 for i=4cc..4cc+4 -> k_bf cols (DVE sums).
                kt1 transposes/casts are emitted later, inside iteration 0,
                so they don't head-block the PE/ScalarE FIFOs."""
                ch = kch_pool.tile([C, 4, 512], f16, tag="kch")
                nc.sync.dma_start(ch[:], fn3[:, 4 * cc : 4 * cc + 4, 256:768])
                ch2 = ch[:].rearrange("c i (r w) -> c i r w", r=2)
                rsum = rs_pool.tile([C, 4, 256], bf16, tag="krs")
                nc.vector.tensor_tensor(rsum[:], ch2[:, :, 0, :], ch2[:, :, 1, :], AOp.add)
                rs4 = rsum[:].rearrange("c j (k f) -> c j k f", f=4)
                kv = k_bf[:, 256 * cc : 256 * cc + 256].rearrange(
                    "c (j k) -> c j k", k=hw
                )
                nc.vector.tensor_tensor(kv, rs4[:, :, :, 1], rs4[:, :, :, 2], AOp.add)

            def make_kt1(cc):
                """Transpose chunk cc's k_bf tiles into kt1 (PE + ScalarE)."""
                for mi in (2 * cc, 2 * cc + 1):
                    pt = ps_b.tile([128, 512], f32, tag="bd")
                    ptb = pt[:, 0:64].bitcast(bf16)  # [128, 128] bf16 view
                    nc.tensor.transpose(
                        ptb, k_bf[:, 128 * mi : 128 * mi + 128], id_t[:]
                    )
                    nc.scalar.copy(kt1[:, mi, 0:128], ptb)

            # ---- band machinery ----
            def band_source(m):
                """(kind, t, matrix name) for band m; kind 'ot' or 'z'."""
                if m == 0:
                    return ("ot", 0, "ET")
                if m == 127:
                    return ("ot", 31, "EB")
                if m % 2 == 0:
                    k = m // 2
                    if k % 2 == 1:
                        return ("ot", (k - 1) // 2, "IN0")
                    return ("z", k // 2 - 1, "ZE")
                k = (m - 1) // 2
                if k % 2 == 0:
                    return ("ot", k // 2, "IN1")
                return ("z", (k - 1) // 2, "ZO")

            def band_stationary(m):
                kind, t, name = band_source(m)
                if kind == "ot":
                    return ot_tiles[t // 4][:, t % 4, :], name
                return zt_tiles[(t + 1) // 4][:, (t + 1) % 4, :], name

            def emit_band(m):
                """Band m = output rows (2m, 2m+1): one PE matmul into PSUM,
                DVE add of fw rows in place (fp16), write block when done."""
                st, name = band_stationary(m)
                bd = ps_b.tile([128, 512], f32, tag="bd")
                nc.tensor.matmul(
                    bd[:], st, bm_t[:, MIDX[name], :], start=True, stop=True
                )
                b = m // 8
                lr = 2 * (m % 8)
                fwb = fw_tiles[b]
                nc.vector.tensor_tensor(
                    fwb[:, lr : lr + 2, :], bd[:], fwb[:, lr : lr + 2, :], AOp.add
                )
                if m % 8 == 7:
                    nc.sync.dma_start(
                        out.ap()[:, 16 * b : 16 * b + 16, :], fw_tiles.pop(b)[:]
                    )

            def norms_and_z(nb):
                """Normalize the 4 sub-tiles of block nb -> ot tile, with the
                Z-tile copies interleaved by dependency so Z_{4nb-1} (slot 0,
                needed first by the band queue) is ready right after the
                first norm.  Z slot s = Z_{4nb-1+s} = [y[2t+1] ; y[2t+2]]."""
                otb = ot_pool.tile([128, 4, 128], bf16, tag="ot", name="ot_t")
                ot_tiles[nb] = otb
                zb = zt_pool.tile([128, 4, 128], bf16, tag="zt", name="zt_t")
                zt_tiles[nb] = zb
                if nb > 0:
                    # slot-0 lower half comes from the previous block
                    nc.sync.dma_start(
                        zb[0:64, 0, :], ot_tiles[nb - 1][64:128, 3, :]
                    )
                pos = pos_of.pop(nb)
                for ns in range(4):
                    po = pos[ns]
                    rcp = sm_pool.tile([128, 1], f32, tag="rcp")
                    nc.vector.reciprocal(rcp[:], po[:, 128:129])
                    nc.vector.tensor_scalar(
                        otb[:, ns, :], po[:, 0:128], rcp[:], None, AOp.mult
                    )
                    if ns == 0:
                        nc.sync.dma_start(zb[64:128, 0, :], otb[0:64, 0, :])
                # remaining slots 1..3
                nc.sync.dma_start(zb[64:128, 1:4, :], otb[0:64, 1:4, :])
                nc.sync.dma_start(zb[0:64, 1:4, :], otb[64:128, 0:3, :])

            # ---- prologue ----
            # Dispatch order on sync tuned for the iteration-0 critical path:
            # kch0/1 gate the first transposes, fw0/1 gate qt(0), bm_t is
            # only needed by the first bands (~iteration 1), fw2/3 only by
            # qx late in iteration 0.
            load_k_chunk(0)
            load_k_chunk(1)
            load_wq(0)
            load_wq(1)
            q_extract_wq(0, nc.gpsimd)  # waits wq0 on gpsimd
            q_extract_wq(1, nc.vector)  # on DVE right behind kch0/1 sums
            load_k_chunk(2)
            load_k_chunk(3)
            for j in range(6):
                nc.sync.dma_start(bm_t[:, j, :], bm.ap()[j, :, :])
            for cc in range(4, 9):
                load_k_chunk(cc)
            load_wq(2)
            load_wq(3)
            for cc in range(9, 16):
                load_k_chunk(cc)
            # full fw blocks 0-3 stream after the k chunks: the band adds
            # only need them from ~iteration 1 on
            load_fw(0)
            load_fw(1)
            load_fw(2)
            load_fw(3)

            bandq = []

            def band_step():
                if bandq:
                    emit_band(bandq.pop(0))

            pos_of = {}

            def spair(g):
                """Scores m-pair: block g//16, pair g%16; exp -> at."""
                nbp, mp = divmod(g, 16)
                qt = q_tiles[nbp]
                ps = ps_s.tile([128, 2, 512], f32, tag="ps")
                for s in range(2):
                    mi = 2 * mp + s
                    nc.tensor.matmul(
                        ps[:, s, :],
                        k_bf[:, 128 * mi : 128 * mi + 128],
                        qt[:],
                        start=True,
                        stop=True,
                    )
                nc.scalar.activation(
                    at[:, 2 * mp : 2 * mp + 2, :],
                    ps[:],
                    ActFn.Exp,
                    bias=0.0,
                    scale=EXP_SCALE,
                )

            def pv_chunk(p):
                """PV accumulation for global pair p (block p//16)."""
                nbp, mp = divmod(p, 16)
                if mp == 0:
                    po0 = ps_o.tile([128, 2, 129], f32, tag="po")
                    po1 = ps_o.tile([128, 2, 129], f32, tag="po")
                    pos_of[nbp] = (
                        po0[:, 0, :], po0[:, 1, :], po1[:, 0, :], po1[:, 1, :]
                    )
                pos = pos_of[nbp]
                for ns in range(4):
                    for mi in (2 * mp, 2 * mp + 1):
                        nc.tensor.matmul(
                            pos[ns],
                            at[:, mi, 128 * ns : 128 * ns + 128],
                            kt1[:, mi, :],
                            start=(mi == 0),
                            stop=(mi == MI - 1),
                        )

            def finish_block(nbf):
                """Norms + Z tiles + enable the band run for block nbf."""
                norms_and_z(nbf)
                # run 16nbf-1 .. 16nbf+14; the two bands on Z_{4nbf-1}
                # (16nbf-1, 16nbf) go behind the first two direct bands so
                # they don't race the Z DMA.  Run 7 is the epilogue's.
                if nbf < NBLK - 1:
                    lo = max(0, 16 * nbf - 1)
                    run = list(range(lo, 16 * nbf + 15))
                    if nbf > 0:
                        run = [run[2], run[3], run[0], run[1]] + run[4:]
                    bandq.extend(run)

            # ---- main loop: one flat 128-slot software pipeline ----
            for g in range(16 * NBLK):
                nb, mp = divmod(g, 16)
                if nb == 0:
                    make_kt1(mp)
                spair(g)
                if g >= 2:
                    pv_chunk(g - 2)
                if mp == 1 and nb >= 1:
                    finish_block(nb - 1)
                if g < 125:
                    band_step()
                if mp == 0 and 2 * nb + 4 < NFB:
                    load_fw(2 * nb + 4)
                if mp == 10 and 2 * nb + 5 < NFB:
                    load_fw(2 * nb + 5)
                qx_slots = (10, 13) if nb == 0 else (5, 11)
                if mp == qx_slots[0] and 2 * nb + 2 < NFB:
                    if nb == 0:
                        q_extract_wq(2, nc.gpsimd)
                    else:
                        q_extract(2 * nb + 2, nc.gpsimd)
                if mp == qx_slots[1] and 2 * nb + 3 < NFB:
                    if nb == 0:
                        q_extract_wq(3, nc.vector)
                    else:
                        q_extract(2 * nb + 3, nc.gpsimd)
            pv_chunk(126)
            pv_chunk(127)
            finish_block(NBLK - 1)
            # held-back bands fill the PE while the last norms and Z tiles
            # are produced
            while bandq:
                emit_band(bandq.pop(0))

            # ---- epilogue: bands 111..127 (tiles 28..31 & Z 27..30).
            # Scores are done, so ps_s banks are free: pair bands into
            # [128,2,512] tiles -> one DVE add per 4 output rows.  The last
            # two blocks are written out in halves so the final DMA overlaps
            # the remaining adds.
            emit_band(111)  # completes block 13
            for m0 in range(112, 128, 2):
                bd2 = ps_s.tile([128, 2, 512], f32, tag="ps")
                for j, m in enumerate((m0, m0 + 1)):
                    st, name = band_stationary(m)
                    nc.tensor.matmul(
                        bd2[:, j, :], st, bm_t[:, MIDX[name], :],
                        start=True, stop=True,
                    )
                b = m0 // 8
                lr = 2 * (m0 % 8)
                fwb = fw_tiles[b]
                nc.vector.tensor_tensor(
                    fwb[:, lr : lr + 4, :], bd2[:], fwb[:, lr : lr + 4, :], AOp.add
                )
                if (m0 + 1) % 8 == 3:
                    nc.sync.dma_start(
                        out.ap()[:, 16 * b : 16 * b + 8, :], fwb[:, 0:8, :]
                    )
                elif (m0 + 1) % 8 == 7:
                    nc.sync.dma_start(
                        out.ap()[:, 16 * b + 8 : 16 * b + 16, :],
                        fw_tiles.pop(b)[:, 8:16, :],
                    )

    nc.compile()
    return nc


_NC_CACHE = None


def _get_nc():
    global _NC_CACHE
    if _NC_CACHE is None:
        _NC_CACHE = build_kernel()
    return _NC_CACHE


def run(feat_wide: np.ndarray, feat_narrow: np.ndarray, trace: bool = False):
    """Run on 8 NeuronCores; returns (output [8,128,256,256], BassKernelResults)."""
    from concourse.bass_utils import run_bass_kernel_spmd
    import ml_dtypes

    B, C, H, W = feat_wide.shape
    assert (B, C, H, W) == (8, 128, 256, 256)

    bandmat = _build_band_matrices().astype(ml_dtypes.bfloat16)
    identity = np.eye(128, dtype=ml_dtypes.bfloat16)

    fw16 = np.asarray(feat_wide, dtype=np.float16)
    fn16 = np.asarray(feat_narrow, dtype=np.float16)

    nc = _get_nc()
    in_maps = [
        {
            "feat_wide": np.ascontiguousarray(fw16[b]),
            "feat_narrow": np.ascontiguousarray(fn16[b]),
            "bandmat": bandmat,
            "ident": identity,
        }
        for b in range(B)
    ]
    res = run_bass_kernel_spmd(nc, in_maps, core_ids=list(range(8)), trace=trace)
    outv = np.stack(
        [np.asarray(res.results[b]["out"], dtype=np.float32) for b in range(B)],
        axis=0,
    )
    return outv, res


def kernel(feat_wide: np.ndarray, feat_narrow: np.ndarray) -> np.ndarray:
    out, _ = run(feat_wide, feat_narrow, trace=False)
    return out
